# revision 47
# baseline (speedup 1.0000x reference)
"""CTAttention Trainium2 kernel.

Full-input contract: kernel(**inputs) takes the unsharded numpy inputs and
returns the full [total, C] output. Internally: data-parallel over the batch
axis B=8 across 8 NeuronCores (attention is independent per batch element);
qkv/proj weights replicated; ragged scatter/gather bookkeeping on the host.

Per-core dataflow (batch b, dense 1024 windows, 8 heads, head_dim 32):
  X^T[256,1024] -> Q^T/K^T (bf16, channel-on-partition) and V[kpos,ch] (bf16)
  attention runs 4 heads (one group) at a time, software-pipelined, with
  c-major (query-chunk-major) iteration so each 512-query half's PV
  accumulation finishes early and its staging/denominator extraction hides
  under the remaining loop:
    scores: S^T = per-head K=32 matmuls, row-packed on the PE array (bf16,
            one 32-row strip per head), into two [128,1024] psums
    exp:    heads 0,1 of the group: ScalarE Exp with the key-padding mask as
            a per-partition bias (masked scores underflow to exactly 0, so no
            row-max pass is needed). heads 2,3: Schraudolph fast-exp on the
            DVE — one tensor_scalar (s*A + maskb) with int16 round-to-nearest
            output whose bits ARE the bf16 P (int16 saturation maps masked
            scores to 0x8000 = -0.0). Splitting whole heads (not key blocks)
            keeps each query's softmax internally consistent, so the ~3%
            fast-exp wiggle largely cancels through the normalization.
    PV:     fused PV+rowsum: stationary is V_h|ones [128,33], so each head's
            matmul emits its O^T strip AND its softmax denominator (psum row
            32/96) from a single moving stream; the two heads of a pair sit
            at PE column tiles 0 and 64.
  normalization: denominators -> 1/x via ScalarE Ln+Exp per 512-query half
  (same ACT table set as Exp/Copy: no table switches), three of four halves
  hidden under the loop; broadcast via K=4 selector matmuls, one DVE multiply
  per pair, then cross-partition strip assembly into head-major channel order
  with SBUF->SBUF DMAs. Output projection consumes the assembled O^T; the
  proj bias is applied host-side; y is returned in bf16.

Exact algebraic simplifications vs the reference:
  - K bias dropped (softmax is invariant to per-query constant shifts)
  - V bias folded into the proj bias (softmax weights sum to 1, applied host-side)
  - head-dim scale folded into the exp's input scale / fast-exp multiplier

Environment workarounds (this walrus build): at most one sem wait per
instruction (waits hoisted onto injected NOPs), fp32/fp32r matmuls require
dst partition base 0, no gpsimd extended instructions (and GpSimd cannot
access PSUM), DMA cannot access PSUM or broadcast (zero-stride APs rejected).
"""

import sys

if "/opt/trn_rl_repo" not in sys.path:
    sys.path.insert(0, "/opt/trn_rl_repo")

import numpy as np

B = 8
C = 256
H = 8
HD = 32
MAXW = 1024
SCALE = HD ** -0.5
NEG_THRESH = -1e8  # mask values below this count as fully masked

_cached = {}


def _build_nc():
    import bass_rust
    import concourse.bass as bass
    import concourse.tile as tile
    import concourse.mybir as mybir
    from concourse.vector_clock import ScopedClock

    # ---- workaround: this walrus build accepts at most ONE sem wait per
    # instruction ("Too many sync wait commands" in setupSyncWait). Tile
    # attaches multi-sem waits freely. Split: hoist all but the last wait of
    # every committed instruction onto injected same-engine NOPs, and split
    # the final drain the same way.
    _ctr = [0]

    def _hoist_excess_waits(tc_self, inst, orig_add):
        si = inst.sync_info
        if si is not None:
            waits = list(si.on_wait or [])
            if len(waits) > 1:
                for w in waits[:-1]:
                    _ctr[0] += 1
                    nop = mybir.InstNoOp(name=f"waitsplit-{_ctr[0]}")
                    nop.engine = inst.engine
                    nop.sync_info = bass_rust.SyncInfo(on_wait=[w], on_update=[])
                    orig_add(tc_self, nop)
                si.on_wait = waits[-1:]
        orig_add(tc_self, inst)

    if not getattr(tile.TileContext, "_waitsplit_patched", False):
        _orig_add_instruction = tile.TileContext._add_instruction

        def _split_add_instruction(self, inst):
            _hoist_excess_waits(self, inst, _orig_add_instruction)

        tile.TileContext._add_instruction = _split_add_instruction

        def _patched_drain_and_barrier(self, tick_clock, wait_clock):
            # Trimmed exit: drain + one barrier. Semaphore clears and the
            # second barrier are dropped — each NEFF execution reinitializes
            # semaphores at program start, so the ~6us of teardown they cost
            # buys nothing (re-execution verified against the first run).
            nc = self.nc
            d0 = nc.sync.drain()
            wait_clock.add_sem_waits(
                d0.ins, ScopedClock({None: tick_clock.global_clock})
            )
            si = d0.ins.sync_info
            waits = list(si.on_wait) if si is not None else []
            if len(waits) > 1:
                si.on_wait = waits[0:1]
                for w in waits[1:]:
                    dk = nc.sync.drain()
                    dk.ins.sync_info = bass_rust.SyncInfo(on_wait=[w], on_update=[])
            nc.all_engine_barrier()
            assert self.sems is not None
            popped = nc._tile_sem_poison_stack.pop()
            assert popped is self._sem_poison

        tile.TileContext._drain_and_barrier = _patched_drain_and_barrier
        tile.TileContext._waitsplit_patched = True

    dt = mybir.dt
    f32 = dt.float32
    f32r = dt.float32r
    i16 = dt.int16
    AF = mybir.ActivationFunctionType
    ALU = mybir.AluOpType

    nc = bass.Bass(
        "TRN2",
        target_bir_lowering=False,
        debug=False,
        num_devices=1,
        enable_asserts=False,
    )

    bf16 = dt.bfloat16

    xt_d = nc.dram_tensor("xt", [128, 2048], bf16, kind="ExternalInput").ap()
    qw_d = nc.dram_tensor("qw", [128, 1536], bf16, kind="ExternalInput").ap()
    qb_d = nc.dram_tensor("qb", [128, 2], f32, kind="ExternalInput").ap()
    pw_d = nc.dram_tensor("pw", [128, 512], bf16, kind="ExternalInput").ap()
    mask_d = nc.dram_tensor("mask", [128, 8], f32, kind="ExternalInput").ap()
    maskb_d = nc.dram_tensor("maskb", [128, 8], f32, kind="ExternalInput").ap()
    sel_d = nc.dram_tensor("sel", [4, 256], bf16, kind="ExternalInput").ap()
    yt_d = nc.dram_tensor("yt", [128, 2048], bf16, kind="ExternalOutput").ap()

    # Schraudolph fast-exp (DVE): bf16(bits) = int16(round(s*FE_A + maskb))
    # where FE_A = SCALE*log2(e)*2^7; int16 saturation maps masked scores to
    # 0x8000 = -0.0. Calibrated bias constant folded into maskb host-side.
    FE_A = float(SCALE * 1.4426950408889634 * 128.0)

    with tile.TileContext(nc) as tc:
        with (
            tc.tile_pool(name="const", bufs=1) as const_pool,
            tc.tile_pool(name="big", bufs=1) as big_pool,
            tc.tile_pool(name="pt", bufs=6) as pt_pool,
            tc.tile_pool(name="stage", bufs=4) as stage_pool,
            tc.tile_pool(name="norm", bufs=2) as norm_pool,
            tc.tile_pool(name="ps_s4a", bufs=1, space="PSUM") as ps_s4a,
            tc.tile_pool(name="ps_s4b", bufs=1, space="PSUM") as ps_s4b,
            tc.tile_pool(name="ps_ot", bufs=2, space="PSUM") as ps_ot,
        ):
            xt = const_pool.tile([128, 2048], bf16, tag="xt")
            qw = const_pool.tile([128, 1536], bf16, tag="qw")
            qb = const_pool.tile([128, 2], f32, tag="qb")
            pw = const_pool.tile([128, 512], bf16, tag="pw")
            mask = const_pool.tile([128, 8], f32, tag="mask")
            maskb = const_pool.tile([128, 8], f32, tag="maskb")
            sel = const_pool.tile([4, 256], bf16, tag="sel")

            # startup DMAs ordered by first consumer: the first qk tiles need
            # xt chunks 0/2 and qw cols [0:384]+[768:1152]; mask gates the
            # first exp. sync/scalar/gpsimd queues run in parallel.
            nc.gpsimd.dma_start(mask[:], mask_d)
            nc.gpsimd.dma_start(maskb[:], maskb_d)
            # qw slices in exact first-consumer order: qk0 needs [0:128]+
            # [768:896], qk2 needs [256:384]+[1024:1152]; tiny first transfers
            # shave the DMA-latency-bound startup chain
            nc.sync.dma_start(xt[:, 0:512], xt_d[:, 0:512])
            nc.scalar.dma_start(qw[:, 0:128], qw_d[:, 0:128])
            nc.sync.dma_start(xt[:, 1024:1536], xt_d[:, 1024:1536])
            nc.scalar.dma_start(qw[:, 768:896], qw_d[:, 768:896])
            nc.scalar.dma_start(qw[:, 256:384], qw_d[:, 256:384])
            nc.scalar.dma_start(qw[:, 1024:1152], qw_d[:, 1024:1152])
            nc.sync.dma_start(xt[:, 512:1024], xt_d[:, 512:1024])
            nc.scalar.dma_start(qw[:, 128:256], qw_d[:, 128:256])
            nc.scalar.dma_start(qw[:, 896:1024], qw_d[:, 896:1024])
            nc.sync.dma_start(xt[:, 1536:2048], xt_d[:, 1536:2048])
            nc.scalar.dma_start(qw[:, 384:768], qw_d[:, 384:768])
            nc.scalar.dma_start(qw[:, 1152:1536], qw_d[:, 1152:1536])
            # dependency-free ACT table warmup (forces the Exp table load now,
            # after the critical qw DMA triggers so it doesn't delay them)
            warm_in = const_pool.tile([1, 2], f32, tag="warm_in")
            nc.vector.memset(warm_in[:], 0.0)
            warm = const_pool.tile([1, 2], f32, tag="warm")
            nc.scalar.activation(warm[:], warm_in[:], AF.Exp, scale=0.0)
            nc.gpsimd.dma_start(qb[:], qb_d)
            nc.gpsimd.dma_start(sel[:], sel_d)
            nc.gpsimd.dma_start(pw[:], pw_d)

            qt = big_pool.tile([128, 2048], bf16, tag="qt")
            kt = big_pool.tile([128, 2048], bf16, tag="kt")
            # [part, kpos_blk, head, head_dim+1]; col 32 = the all-ones column
            # that makes each PV matmul emit the softmax denominator as an
            # extra psum row (stationary = V_h | ones -> out rows 0-31 = O^T
            # strip, row 32 = rowsum)
            va = big_pool.tile([128, 8, 8, 33], bf16, tag="va")
            otf = big_pool.tile([128, 2048], bf16, tag="otf")
            ytile = big_pool.tile([128, 2048], bf16, tag="ytile")
            nc.vector.memset(va[:, :, :, 32:33], 1.0)

            # ---------- qkv projections ----------
            # V first (PV needs all of it), then the Q/K tiles needed by the
            # first head group; the rest is emitted between the groups so the
            # PE fills ACT-bound gaps.
            def qk_tile(m, chunks=(0, 1), pool=None, ptag="ot"):
                pool = pool if pool is not None else ps_ot
                for c in chunks:
                    ps = pool.tile([128, 512], f32, tag=ptag, name=f"qk{m}{c}")
                    for t in range(2):
                        nc.tensor.matmul(
                            ps[:],
                            qw[:, 768 * t + 128 * m : 768 * t + 128 * (m + 1)],
                            xt[:, 1024 * t + 512 * c : 1024 * t + 512 * (c + 1)],
                            start=(t == 0),
                            stop=(t == 1),
                        )
                    if m < 2:
                        nc.vector.tensor_scalar_add(
                            qt[:, 1024 * m + 512 * c : 1024 * m + 512 * (c + 1)],
                            ps[:],
                            qb[:, m : m + 1],
                        )
                    else:
                        nc.vector.tensor_copy(
                            kt[:, 1024 * (m - 2) + 512 * c : 1024 * (m - 2) + 512 * (c + 1)],
                            ps[:],
                        )

            # V: out[kpos_block, cv] in bf16 (no bias; folded into proj bias)
            def v_tile(j, pool=None, ptag="ot"):
                pool = pool if pool is not None else ps_ot
                ps = pool.tile([128, 256], f32, tag=ptag, name=f"v{j}")
                for t in range(2):
                    nc.tensor.matmul(
                        ps[:],
                        xt[:, 1024 * t + 128 * j : 1024 * t + 128 * (j + 1)],
                        qw[:, 768 * t + 512 : 768 * t + 768],
                        start=(t == 0),
                        stop=(t == 1),
                    )
                nc.vector.tensor_copy(
                    va[:, j, :, 0:32],
                    ps[:].rearrange("p (h d) -> p h d", d=32),
                )

            # only the chunk-0 Q/K tiles gate the first score matmul; the rest
            # of the projections are emitted inside the first iterations
            qk_tile(0, chunks=(0,))
            qk_tile(2, chunks=(0,))

            # ---------- attention: 4 heads (one group) at a time ----------
            # scores: 4-way row-packed fp32r matmuls, two [128,1024] psum tiles
            # per step (double-buffered so ScalarE exps run back-to-back), one
            # exp per tile -> bf16 P^T, then 8 bf16 PV/rowsum matmuls col-tiled
            # across the 4 PE column strips.
            # O^T strips: head hh at psum rows 32*hh of its pair psum
            # (pair 0 = hh 0,1 rows 0-63; pair 1 = hh 2,3 rows 64-127);
            # denominator rows: hh -> (64, 96, 0, 32).
            saved = {}
            for grp in range(2):
                ov0 = ps_ot.tile([128, 1024], f32, tag="ot", name=f"ov0_{grp}")
                ov1 = ps_ot.tile([128, 1024], f32, tag="ot", name=f"ov1_{grp}")

                def emit_scores(j, c):
                    s4a = ps_s4a.tile([128, 1024], f32, tag="s4a", name=f"s4a{grp}{j}{c}")
                    s4b = ps_s4b.tile([128, 1024], f32, tag="s4b", name=f"s4b{grp}{j}{c}")
                    for hh in range(4):
                        s4 = s4a if hh < 2 else s4b
                        base = 32 * hh
                        nc.tensor.matmul(
                            s4[:, 512 * (hh % 2) : 512 * (hh % 2 + 1)],
                            kt[base : base + 32,
                               1024 * grp + 128 * j : 1024 * grp + 128 * (j + 1)],
                            qt[base : base + 32,
                               1024 * grp + 512 * c : 1024 * grp + 512 * (c + 1)],
                            start=True,
                            stop=True,
                            tile_position=(base, 0),
                        )
                    pta = pt_pool.tile([128, 1024], bf16, tag="pt", name=f"pta{grp}{j}{c}")
                    ptb = pt_pool.tile([128, 1024], bf16, tag="pt", name=f"ptb{grp}{j}{c}")
                    nc.scalar.activation(
                        pta[:], s4a[:], AF.Exp, bias=mask[:, j : j + 1], scale=SCALE,
                    )
                    # heads 2,3 of the group take the DVE fast-exp: halves the
                    # ScalarE stream and releases both score psums in parallel
                    # so all four score matmuls can row-pack concurrently
                    nc.vector.tensor_scalar(
                        ptb[:].bitcast(i16), s4b[:], FE_A, maskb[:, j : j + 1],
                        ALU.mult, ALU.add,
                    )
                    return pta, ptb

                def emit_pv(pta, ptb, j, c):
                    sj = (j == 0)
                    ej = (j == 7)
                    # fused PV+rowsum: stationary is V_h | ones [128, 33], so
                    # each head's matmul emits its O^T strip (rows 0-31 of the
                    # 33-row block) AND its softmax denominator (row 32) from a
                    # single moving stream. 33 output rows round up to a 64-col
                    # PE tile, so the two heads of a pair sit at tile columns 0
                    # and 64 (psum rows 0-32 / 64-96).
                    for hh in range(4):
                        h = 4 * grp + hh
                        ov = ov0 if hh < 2 else ov1
                        pt = pta if hh < 2 else ptb
                        vpos = 64 * (hh % 2)
                        nc.tensor.matmul(
                            ov[vpos : vpos + 33, 512 * c : 512 * (c + 1)],
                            va[:, j, h, :],
                            pt[:, 512 * (hh % 2) : 512 * (hh % 2 + 1)],
                            start=sj,
                            stop=ej,
                            tile_position=(0, vpos),
                        )

                st0 = stage_pool.tile([128, 1024], f32, tag="st", name=f"st0_{grp}")
                st1 = stage_pool.tile([128, 1024], f32, tag="st", name=f"st1_{grp}")
                se4 = norm_pool.tile([4, 1024], f32, tag="se4", name=f"se4_{grp}")
                rc4 = norm_pool.tile([4, 1024], bf16, tag="rc4", name=f"rc4_{grp}")

                def emit_half_tail(c):
                    # the c-major order finishes each 512-query half's PV
                    # accumulation 8 iterations before the grp ends: stage it
                    # and extract its denominators while the loop continues.
                    # With the fused PV+rowsum, denominators live at psum rows
                    # 32 (even head) and 96 (odd head) of each pair psum.
                    sl = slice(512 * c, 512 * (c + 1))
                    last = grp == 1 and c == 1
                    # staging split across ScalarE and DVE so neither exp
                    # stream takes the full 1.4us boundary insert
                    nc.scalar.copy(st0[:, sl], ov0[:, sl])
                    nc.vector.tensor_copy(st1[:, sl], ov1[:, sl])
                    nc.sync.dma_start(se4[0:1, sl], st0[32:33, sl])
                    nc.gpsimd.dma_start(se4[1:2, sl], st0[96:97, sl])
                    nc.sync.dma_start(se4[2:3, sl], st1[32:33, sl])
                    nc.scalar.dma_start(se4[3:4, sl], st1[96:97, sl])
                    if not last:
                        # reciprocal for this half on ScalarE (Ln + Exp share
                        # the loaded table set): ~1.3us riding the exp stream's
                        # slack, so the exit tail only handles the final half
                        lnh = norm_pool.tile([4, 512], f32, tag="ln4", name=f"ln_{grp}{c}")
                        nc.scalar.activation(lnh[:], se4[:, sl], AF.Ln)
                        nc.scalar.activation(rc4[:, sl], lnh[:], AF.Exp, scale=-1.0)

                iters = [(j, c) for c in range(2) for j in range(8)]
                pend = None
                for idx in range(len(iters) + 1):
                    if idx < len(iters):
                        j, c = iters[idx]
                        cur = (*emit_scores(j, c), j, c)
                        if grp == 0 and idx == 0:
                            # V projections and the remaining Q/K tiles fill the
                            # first exp latencies; emitted before any PV so the
                            # ov accumulators don't yet hold the psum slots
                            v_tile(0)
                            v_tile(1)
                            for jj in range(2, 6):
                                v_tile(jj)
                        if grp == 0 and idx == 1:
                            v_tile(6)
                            v_tile(7)
                            qk_tile(0, chunks=(1,))
                            qk_tile(2, chunks=(1,))
                            qk_tile(1)
                            qk_tile(3)
                    else:
                        cur = None
                    if pend is not None:
                        emit_pv(*pend)
                        if pend[2] == 7:
                            emit_half_tail(pend[3])
                    pend = cur

                # ---- last-half reciprocal (the other halves ran mid-loop) ----
                if grp == 1:
                    ln4 = norm_pool.tile([4, 512], f32, tag="ln4", name=f"ln4_{grp}")
                    nc.scalar.activation(ln4[:], se4[:, 512:1024], AF.Ln)
                    nc.scalar.activation(rc4[:, 512:1024], ln4[:], AF.Exp, scale=-1.0)
                saved[grp] = (st0, st1, rc4)

            # ---- deferred normalization: broadcast 1/denominator and scale ----
            # bca rows 0-31 = 1/d(h_even), rows 64-95 = 1/d(h_odd) (matches the
            # staged psum layout); bcb likewise for the st1 pair. The nx tiles
            # are then strip-assembled into otf's head-major channel order with
            # cross-partition SBUF->SBUF DMAs (DMA is the only engine that can
            # move data across partitions). grp1 is split per 512-query half so
            # the c0 chain (reciprocal precomputed mid-loop) drains while the
            # c1 chain waits on the post-loop Ln+Exp.
            bct = {}
            ntiles = {}
            for grp in range(2):
                st0, st1, rc4 = saved[grp]
                bca = ps_s4a.tile([128, 1024], f32, tag="s4a", name=f"bca{grp}")
                bcb = ps_s4b.tile([128, 1024], f32, tag="s4b", name=f"bcb{grp}")
                bct[grp] = (bca, bcb)
                ntiles[grp] = (
                    pt_pool.tile([128, 1024], bf16, tag="pt", name=f"n0_{grp}"),
                    pt_pool.tile([128, 1024], bf16, tag="pt", name=f"n1_{grp}"),
                )

            def emit_norm(grp, cs):
                st0, st1, rc4 = saved[grp]
                bca, bcb = bct[grp]
                n0, n1 = ntiles[grp]
                sl = slice(512 * cs[0], 512 * (cs[-1] + 1))
                for c in cs:
                    csl = slice(512 * c, 512 * (c + 1))
                    nc.tensor.matmul(
                        bca[:, csl], sel[:, 0:128], rc4[:, csl], start=True, stop=True
                    )
                    nc.tensor.matmul(
                        bcb[:, csl], sel[:, 128:256], rc4[:, csl], start=True, stop=True
                    )
                nc.vector.tensor_mul(n0[:, sl], st0[:, sl], bca[:, sl])
                nc.vector.tensor_mul(n1[:, sl], st1[:, sl], bcb[:, sl])
                gb = slice(1024 * grp + sl.start, 1024 * grp + sl.stop)
                # three parallel trigger queues so all four strips land ~together
                eng0 = nc.sync if grp == 0 else nc.scalar
                eng0.dma_start(otf[0:32, gb], n0[0:32, sl])
                nc.gpsimd.dma_start(otf[32:64, gb], n0[64:96, sl])
                nc.sync.dma_start(otf[64:96, gb], n1[0:32, sl])
                nc.gpsimd.dma_start(otf[96:128, gb], n1[64:96, sl])

            # projection accumulation is split: the t=0 contraction half reads
            # grp0's otf block (assembled early), so the PE fills what would
            # otherwise be a dead wait on the grp1 normalization chains
            proj_ps = {}

            def emit_proj_t0(c):
                for m in range(2):
                    ps = ps_ot.tile([128, 512], f32, tag="ot", name=f"proj{c}{m}")
                    proj_ps[(c, m)] = ps
                    nc.tensor.matmul(
                        ps[:],
                        pw[:, 128 * m : 128 * (m + 1)],
                        otf[:, 512 * c : 512 * (c + 1)],
                        start=True,
                        stop=False,
                    )

            def emit_proj_t1(c):
                for m in range(2):
                    ps = proj_ps[(c, m)]
                    nc.tensor.matmul(
                        ps[:],
                        pw[:, 256 + 128 * m : 256 + 128 * (m + 1)],
                        otf[:, 1024 + 512 * c : 1024 + 512 * (c + 1)],
                        start=False,
                        stop=True,
                    )
                    ysl = slice(1024 * m + 512 * c, 1024 * m + 512 * (c + 1))
                    # proj bias is applied host-side; c1 copies ride the
                    # by-then-idle ScalarE so DVE isn't the tail bottleneck
                    (nc.vector.tensor_copy if c == 0 else nc.scalar.copy)(
                        ytile[:, ysl], ps[:]
                    )
                    (nc.sync if c == 0 else nc.gpsimd).dma_start(
                        yt_d[:, ysl], ytile[:, ysl]
                    )

            emit_norm(0, (0, 1))
            emit_proj_t0(0)
            emit_norm(1, (0,))
            emit_proj_t1(0)
            emit_norm(1, (1,))
            emit_proj_t0(1)
            emit_proj_t1(1)

    return nc


def _get_nc():
    if "nc" not in _cached:
        _cached["nc"] = _build_nc()
    return _cached["nc"]


def _pack_per_partition(a2d):
    """[2*128, F] -> [128, 2*F] with tile t at cols F*t."""
    n, f = a2d.shape
    t = n // 128
    return np.ascontiguousarray(
        a2d.reshape(t, 128, f).transpose(1, 0, 2).reshape(128, t * f)
    )


def _prepare(carrier_tokens, ct_mask, batch_num_windows, qkv_w, qkv_b, proj_w, proj_b):
    """Host-side bookkeeping: ragged->padded scatter, weight packing.
    Returns (in_maps, ctx) where ctx carries what postprocessing needs."""
    carrier_tokens = np.asarray(carrier_tokens, dtype=np.float32)
    ct_mask = np.asarray(ct_mask, dtype=np.float32)
    lens = np.asarray(batch_num_windows).astype(np.int64)
    qkv_w = np.asarray(qkv_w, dtype=np.float32)
    qkv_b = np.asarray(qkv_b, dtype=np.float32)
    proj_w = np.asarray(proj_w, dtype=np.float32)
    proj_b = np.asarray(proj_b, dtype=np.float32)

    total = carrier_tokens.shape[0]

    # ragged -> padded bookkeeping (mirrors the reference's scatter semantics:
    # OOB scatter indices dropped, OOB gather indices clipped)
    offsets = np.concatenate([[0], np.cumsum(lens)])
    tok = np.arange(total)
    b_id = np.searchsorted(offsets[1:], tok, side="right")
    w_id = tok - offsets[np.minimum(b_id, B)]
    flat_idx = b_id * MAXW + w_id
    valid = flat_idx < B * MAXW
    padded = np.zeros((B * MAXW, C), np.float32)
    padded[flat_idx[valid]] = carrier_tokens[valid]
    padded = padded.reshape(B, MAXW, C)

    mask_col = np.ascontiguousarray(ct_mask[:, 0, :])  # [B, MAXW]

    # host-side exact weight transforms; the effective proj bias (V bias
    # folded in via softmax-sums-to-1) is applied host-side in _postprocess
    pb_eff = qkv_b[2 * C : 3 * C] @ proj_w + proj_b

    qw_packed = _pack_per_partition(qkv_w)                      # [128, 1536]
    qb_packed = np.ascontiguousarray(qkv_b[0:C].reshape(2, 128).T)
    pw_packed = _pack_per_partition(proj_w)                     # [128, 512]

    import ml_dtypes
    # sel[:, 0:128] broadcasts 1/d of the even/odd head of the st0 pair to
    # rows 0-31 / 64-95; sel[:, 128:256] likewise for the st1 pair
    sel_arr = np.zeros((4, 256), ml_dtypes.bfloat16)
    sel_arr[0, 0:32] = 1.0
    sel_arr[1, 64:96] = 1.0
    sel_arr[2, 128:160] = 1.0
    sel_arr[3, 192:224] = 1.0
    qw_packed = qw_packed.astype(ml_dtypes.bfloat16)
    pw_packed = pw_packed.astype(ml_dtypes.bfloat16)
    # fast-exp per-partition bias: maskb = mask*log2(e)*2^7 + (2^7*127 - c)
    # with the calibrated correction c = 11.5 (int16 round-to-nearest)
    LOG2E = 1.4426950408889634
    FE_B0 = np.float32(16256.0 - 11.5)
    in_maps = []
    for b in range(B):
        xt = _pack_per_partition(padded[b].T).astype(ml_dtypes.bfloat16)
        mb = np.ascontiguousarray(mask_col[b].reshape(8, 128).T)
        mbb = (mb * np.float32(LOG2E * 128.0) + FE_B0).astype(np.float32)
        in_maps.append(
            {
                "xt": xt,
                "qw": qw_packed,
                "qb": qb_packed,
                "pw": pw_packed,
                "mask": mb,
                "maskb": mbb,
                "sel": sel_arr,
            }
        )

    ctx = {
        "flat_idx": flat_idx,
        "mask_col": mask_col,
        "padded": padded,
        "qkv_w": qkv_w,
        "qkv_b": qkv_b,
        "proj_w": proj_w,
        "proj_b": proj_b,
        "pb_eff": pb_eff,
    }
    return in_maps, ctx


def _postprocess(results, ctx):
    """Per-core outputs -> full ragged output (gather + degenerate-row fix)."""
    flat_idx = ctx["flat_idx"]
    mask_col = ctx["mask_col"]
    padded = ctx["padded"]
    qkv_w, qkv_b = ctx["qkv_w"], ctx["qkv_b"]
    proj_w, proj_b = ctx["proj_w"], ctx["proj_b"]

    y_pad = np.empty((B, MAXW, C), np.float32)
    for b in range(B):
        yt = np.asarray(results[b]["yt"], dtype=np.float32)     # [128, 2048]
        y_t = yt.reshape(128, 2, MAXW).transpose(1, 0, 2).reshape(C, MAXW)
        y_pad[b] = y_t.T
    y_flat = y_pad.reshape(B * MAXW, C)
    gather_idx = np.clip(flat_idx, 0, B * MAXW - 1)
    # the effective proj bias is applied here (host side) rather than on-chip
    out = y_flat[gather_idx] + ctx["pb_eff"].astype(np.float32)[None, :]

    # degenerate rows: gathered positions whose key mask is fully masked.
    # The reference's softmax (with max-subtraction) gives uniform weights
    # there; our exp underflows to 0/0. Recompute those rows exactly.
    row_b = np.minimum(gather_idx // MAXW, B - 1)
    degenerate_batches = [b for b in range(B) if np.all(mask_col[b] < NEG_THRESH)]
    for b in degenerate_batches:
        rows = np.nonzero(row_b == b)[0]
        if rows.size == 0:
            continue
        vmat = padded[b] @ qkv_w[:, 2 * C : 3 * C] + qkv_b[2 * C : 3 * C]
        mean_v = vmat.mean(axis=0)  # uniform attention, same for all heads
        fix = mean_v @ proj_w + proj_b
        out[rows] = fix.astype(np.float32)

    return np.ascontiguousarray(out.astype(np.float32))


def run_device(in_maps, **spmd_kwargs):
    from concourse import bass_utils

    nc = _get_nc()
    return bass_utils.run_bass_kernel_spmd(
        nc, in_maps, core_ids=list(range(B)), **spmd_kwargs
    )


def kernel(carrier_tokens, ct_mask, batch_num_windows, qkv_w, qkv_b, proj_w, proj_b):
    in_maps, ctx = _prepare(
        carrier_tokens, ct_mask, batch_num_windows, qkv_w, qkv_b, proj_w, proj_b
    )
    res = run_device(in_maps, trace=False)
    return _postprocess(res.results, ctx)



# revision 50
# speedup vs baseline: 1.0004x; 1.0004x over previous
"""CTAttention Trainium2 kernel.

Full-input contract: kernel(**inputs) takes the unsharded numpy inputs and
returns the full [total, C] output. Internally: data-parallel over the batch
axis B=8 across 8 NeuronCores (attention is independent per batch element);
qkv/proj weights replicated; ragged scatter/gather bookkeeping on the host.

Per-core dataflow (batch b, dense 1024 windows, 8 heads, head_dim 32):
  X^T[256,1024] -> Q^T/K^T (bf16, channel-on-partition) and V[kpos,ch] (bf16)
  attention runs 4 heads (one group) at a time, software-pipelined, with
  c-major (query-chunk-major) iteration so each 512-query half's PV
  accumulation finishes early and its staging/denominator extraction hides
  under the remaining loop:
    scores: S^T = per-head K=32 matmuls, row-packed on the PE array (bf16,
            one 32-row strip per head), into two [128,1024] psums
    exp:    heads 0,1 of the group: ScalarE Exp with the key-padding mask as
            a per-partition bias (masked scores underflow to exactly 0, so no
            row-max pass is needed). heads 2,3: Schraudolph fast-exp on the
            DVE — one tensor_scalar (s*A + maskb) with int16 round-to-nearest
            output whose bits ARE the bf16 P (int16 saturation maps masked
            scores to 0x8000 = -0.0). Splitting whole heads (not key blocks)
            keeps each query's softmax internally consistent, so the ~3%
            fast-exp wiggle largely cancels through the normalization.
    PV:     fused PV+rowsum: stationary is V_h|ones [128,33], so each head's
            matmul emits its O^T strip AND its softmax denominator (psum row
            32/96) from a single moving stream; the two heads of a pair sit
            at PE column tiles 0 and 64.
  normalization: denominators -> 1/x via ScalarE Ln+Exp per 512-query half
  (same ACT table set as Exp/Copy: no table switches), three of four halves
  hidden under the loop; broadcast via K=4 selector matmuls, one DVE multiply
  per pair, then cross-partition strip assembly into head-major channel order
  with SBUF->SBUF DMAs. Output projection consumes the assembled O^T; the
  proj bias is applied host-side; y is returned in bf16.

Exact algebraic simplifications vs the reference:
  - K bias dropped (softmax is invariant to per-query constant shifts)
  - V bias folded into the proj bias (softmax weights sum to 1, applied host-side)
  - head-dim scale folded into the exp's input scale / fast-exp multiplier

Environment workarounds (this walrus build): at most one sem wait per
instruction (waits hoisted onto injected NOPs), fp32/fp32r matmuls require
dst partition base 0, no gpsimd extended instructions (and GpSimd cannot
access PSUM), DMA cannot access PSUM or broadcast (zero-stride APs rejected).
"""

import sys

if "/opt/trn_rl_repo" not in sys.path:
    sys.path.insert(0, "/opt/trn_rl_repo")

import numpy as np

B = 8
C = 256
H = 8
HD = 32
MAXW = 1024
SCALE = HD ** -0.5
NEG_THRESH = -1e8  # mask values below this count as fully masked

_cached = {}


def _build_nc():
    import bass_rust
    import concourse.bass as bass
    import concourse.tile as tile
    import concourse.mybir as mybir
    from concourse.vector_clock import ScopedClock

    # ---- workaround: this walrus build accepts at most ONE sem wait per
    # instruction ("Too many sync wait commands" in setupSyncWait). Tile
    # attaches multi-sem waits freely. Split: hoist all but the last wait of
    # every committed instruction onto injected same-engine NOPs, and split
    # the final drain the same way.
    _ctr = [0]

    def _hoist_excess_waits(tc_self, inst, orig_add):
        si = inst.sync_info
        if si is not None:
            waits = list(si.on_wait or [])
            if len(waits) > 1:
                for w in waits[:-1]:
                    _ctr[0] += 1
                    nop = mybir.InstNoOp(name=f"waitsplit-{_ctr[0]}")
                    nop.engine = inst.engine
                    nop.sync_info = bass_rust.SyncInfo(on_wait=[w], on_update=[])
                    orig_add(tc_self, nop)
                si.on_wait = waits[-1:]
        orig_add(tc_self, inst)

    if not getattr(tile.TileContext, "_waitsplit_patched", False):
        _orig_add_instruction = tile.TileContext._add_instruction

        def _split_add_instruction(self, inst):
            _hoist_excess_waits(self, inst, _orig_add_instruction)

        tile.TileContext._add_instruction = _split_add_instruction

        def _patched_drain_and_barrier(self, tick_clock, wait_clock):
            # Trimmed exit: drain + one barrier. Semaphore clears and the
            # second barrier are dropped — each NEFF execution reinitializes
            # semaphores at program start, so the ~6us of teardown they cost
            # buys nothing (re-execution verified against the first run).
            nc = self.nc
            d0 = nc.sync.drain()
            wait_clock.add_sem_waits(
                d0.ins, ScopedClock({None: tick_clock.global_clock})
            )
            si = d0.ins.sync_info
            waits = list(si.on_wait) if si is not None else []
            if len(waits) > 1:
                si.on_wait = waits[0:1]
                for w in waits[1:]:
                    dk = nc.sync.drain()
                    dk.ins.sync_info = bass_rust.SyncInfo(on_wait=[w], on_update=[])
            nc.all_engine_barrier()
            assert self.sems is not None
            popped = nc._tile_sem_poison_stack.pop()
            assert popped is self._sem_poison

        tile.TileContext._drain_and_barrier = _patched_drain_and_barrier
        tile.TileContext._waitsplit_patched = True

    dt = mybir.dt
    f32 = dt.float32
    f32r = dt.float32r
    i16 = dt.int16
    AF = mybir.ActivationFunctionType
    ALU = mybir.AluOpType

    nc = bass.Bass(
        "TRN2",
        target_bir_lowering=False,
        debug=False,
        num_devices=1,
        enable_asserts=False,
    )

    bf16 = dt.bfloat16

    xt_d = nc.dram_tensor("xt", [128, 2048], bf16, kind="ExternalInput").ap()
    qw_d = nc.dram_tensor("qw", [128, 1536], bf16, kind="ExternalInput").ap()
    qb_d = nc.dram_tensor("qb", [128, 2], f32, kind="ExternalInput").ap()
    pw_d = nc.dram_tensor("pw", [128, 512], bf16, kind="ExternalInput").ap()
    mask_d = nc.dram_tensor("mask", [128, 8], f32, kind="ExternalInput").ap()
    maskb_d = nc.dram_tensor("maskb", [128, 8], f32, kind="ExternalInput").ap()
    sel_d = nc.dram_tensor("sel", [4, 256], bf16, kind="ExternalInput").ap()
    yt_d = nc.dram_tensor("yt", [128, 2048], bf16, kind="ExternalOutput").ap()

    # Schraudolph fast-exp (DVE): bf16(bits) = int16(round(s*FE_A + maskb))
    # where FE_A = SCALE*log2(e)*2^7; int16 saturation maps masked scores to
    # 0x8000 = -0.0. Calibrated bias constant folded into maskb host-side.
    FE_A = float(SCALE * 1.4426950408889634 * 128.0)

    with tile.TileContext(nc) as tc:
        with (
            tc.tile_pool(name="const", bufs=1) as const_pool,
            tc.tile_pool(name="big", bufs=1) as big_pool,
            tc.tile_pool(name="pt", bufs=6) as pt_pool,
            tc.tile_pool(name="stage", bufs=4) as stage_pool,
            tc.tile_pool(name="norm", bufs=2) as norm_pool,
            tc.tile_pool(name="ps_s4a", bufs=1, space="PSUM") as ps_s4a,
            tc.tile_pool(name="ps_s4b", bufs=1, space="PSUM") as ps_s4b,
            tc.tile_pool(name="ps_ot", bufs=2, space="PSUM") as ps_ot,
        ):
            xt = const_pool.tile([128, 2048], bf16, tag="xt")
            qw = const_pool.tile([128, 1536], bf16, tag="qw")
            qb = const_pool.tile([128, 2], f32, tag="qb")
            pw = const_pool.tile([128, 512], bf16, tag="pw")
            mask = const_pool.tile([128, 8], f32, tag="mask")
            maskb = const_pool.tile([128, 8], f32, tag="maskb")
            sel = const_pool.tile([4, 256], bf16, tag="sel")

            # startup DMAs ordered by first consumer: the first qk tiles need
            # xt chunks 0/2 and qw cols [0:384]+[768:1152]; mask gates the
            # first exp. sync/scalar/gpsimd queues run in parallel.
            nc.gpsimd.dma_start(mask[:], mask_d)
            nc.gpsimd.dma_start(maskb[:], maskb_d)
            # qw slices in exact first-consumer order: qk0 needs [0:128]+
            # [768:896], qk2 needs [256:384]+[1024:1152]; tiny first transfers
            # shave the DMA-latency-bound startup chain
            nc.sync.dma_start(xt[:, 0:512], xt_d[:, 0:512])
            nc.scalar.dma_start(qw[:, 0:128], qw_d[:, 0:128])
            nc.sync.dma_start(xt[:, 1024:1536], xt_d[:, 1024:1536])
            nc.scalar.dma_start(qw[:, 768:896], qw_d[:, 768:896])
            nc.scalar.dma_start(qw[:, 256:384], qw_d[:, 256:384])
            nc.scalar.dma_start(qw[:, 1024:1152], qw_d[:, 1024:1152])
            nc.sync.dma_start(xt[:, 512:1024], xt_d[:, 512:1024])
            nc.scalar.dma_start(qw[:, 128:256], qw_d[:, 128:256])
            nc.scalar.dma_start(qw[:, 896:1024], qw_d[:, 896:1024])
            nc.sync.dma_start(xt[:, 1536:2048], xt_d[:, 1536:2048])
            nc.scalar.dma_start(qw[:, 384:768], qw_d[:, 384:768])
            nc.scalar.dma_start(qw[:, 1152:1536], qw_d[:, 1152:1536])
            # dependency-free ACT table warmup (forces the Exp table load now,
            # after the critical qw DMA triggers so it doesn't delay them)
            warm_in = const_pool.tile([1, 2], f32, tag="warm_in")
            nc.vector.memset(warm_in[:], 0.0)
            warm = const_pool.tile([1, 2], f32, tag="warm")
            nc.scalar.activation(warm[:], warm_in[:], AF.Exp, scale=0.0)
            nc.gpsimd.dma_start(qb[:], qb_d)
            nc.gpsimd.dma_start(sel[:], sel_d)
            nc.gpsimd.dma_start(pw[:], pw_d)

            qt = big_pool.tile([128, 2048], bf16, tag="qt")
            kt = big_pool.tile([128, 2048], bf16, tag="kt")
            # [part, kpos_blk, head, head_dim+1]; col 32 = the all-ones column
            # that makes each PV matmul emit the softmax denominator as an
            # extra psum row (stationary = V_h | ones -> out rows 0-31 = O^T
            # strip, row 32 = rowsum)
            va = big_pool.tile([128, 8, 8, 33], bf16, tag="va")
            otf = big_pool.tile([128, 2048], bf16, tag="otf")
            ytile = big_pool.tile([128, 2048], bf16, tag="ytile")
            nc.vector.memset(va[:, :, :, 32:33], 1.0)

            # ---------- qkv projections ----------
            # V first (PV needs all of it), then the Q/K tiles needed by the
            # first head group; the rest is emitted between the groups so the
            # PE fills ACT-bound gaps.
            def qk_tile(m, chunks=(0, 1), pool=None, ptag="ot"):
                pool = pool if pool is not None else ps_ot
                for c in chunks:
                    ps = pool.tile([128, 512], f32, tag=ptag, name=f"qk{m}{c}")
                    for t in range(2):
                        nc.tensor.matmul(
                            ps[:],
                            qw[:, 768 * t + 128 * m : 768 * t + 128 * (m + 1)],
                            xt[:, 1024 * t + 512 * c : 1024 * t + 512 * (c + 1)],
                            start=(t == 0),
                            stop=(t == 1),
                        )
                    # qt/kt consumers ride ScalarE (AF.Copy applies scale+bias,
                    # same ACT table set): during the qkv phase the DVE is busy
                    # with fast-exps, and these copies gate the psum pool's WAR
                    # release for the next qkv matmul
                    if m < 2:
                        nc.scalar.activation(
                            qt[:, 1024 * m + 512 * c : 1024 * m + 512 * (c + 1)],
                            ps[:],
                            AF.Identity,
                            bias=qb[:, m : m + 1],
                            scale=1.0,
                        )
                    else:
                        nc.scalar.copy(
                            kt[:, 1024 * (m - 2) + 512 * c : 1024 * (m - 2) + 512 * (c + 1)],
                            ps[:],
                        )

            # V: out[kpos_block, cv] in bf16 (no bias; folded into proj bias)
            def v_tile(j, pool=None, ptag="ot"):
                pool = pool if pool is not None else ps_ot
                ps = pool.tile([128, 256], f32, tag=ptag, name=f"v{j}")
                for t in range(2):
                    nc.tensor.matmul(
                        ps[:],
                        xt[:, 1024 * t + 128 * j : 1024 * t + 128 * (j + 1)],
                        qw[:, 768 * t + 512 : 768 * t + 768],
                        start=(t == 0),
                        stop=(t == 1),
                    )
                nc.vector.tensor_copy(
                    va[:, j, :, 0:32],
                    ps[:].rearrange("p (h d) -> p h d", d=32),
                )

            # only the chunk-0 Q/K tiles gate the first score matmul; the rest
            # of the projections are emitted inside the first iterations
            qk_tile(0, chunks=(0,))
            qk_tile(2, chunks=(0,))

            # ---------- attention: 4 heads (one group) at a time ----------
            # scores: 4-way row-packed fp32r matmuls, two [128,1024] psum tiles
            # per step (double-buffered so ScalarE exps run back-to-back), one
            # exp per tile -> bf16 P^T, then 8 bf16 PV/rowsum matmuls col-tiled
            # across the 4 PE column strips.
            # O^T strips: head hh at psum rows 32*hh of its pair psum
            # (pair 0 = hh 0,1 rows 0-63; pair 1 = hh 2,3 rows 64-127);
            # denominator rows: hh -> (64, 96, 0, 32).
            saved = {}
            for grp in range(2):
                ov0 = ps_ot.tile([128, 1024], f32, tag="ot", name=f"ov0_{grp}")
                ov1 = ps_ot.tile([128, 1024], f32, tag="ot", name=f"ov1_{grp}")

                def emit_scores(j, c):
                    s4a = ps_s4a.tile([128, 1024], f32, tag="s4a", name=f"s4a{grp}{j}{c}")
                    s4b = ps_s4b.tile([128, 1024], f32, tag="s4b", name=f"s4b{grp}{j}{c}")
                    for hh in range(4):
                        s4 = s4a if hh < 2 else s4b
                        base = 32 * hh
                        nc.tensor.matmul(
                            s4[:, 512 * (hh % 2) : 512 * (hh % 2 + 1)],
                            kt[base : base + 32,
                               1024 * grp + 128 * j : 1024 * grp + 128 * (j + 1)],
                            qt[base : base + 32,
                               1024 * grp + 512 * c : 1024 * grp + 512 * (c + 1)],
                            start=True,
                            stop=True,
                            tile_position=(base, 0),
                        )
                    pta = pt_pool.tile([128, 1024], bf16, tag="pt", name=f"pta{grp}{j}{c}")
                    ptb = pt_pool.tile([128, 1024], bf16, tag="pt", name=f"ptb{grp}{j}{c}")
                    nc.scalar.activation(
                        pta[:], s4a[:], AF.Exp, bias=mask[:, j : j + 1], scale=SCALE,
                    )
                    # heads 2,3 of the group take the DVE fast-exp: halves the
                    # ScalarE stream and releases both score psums in parallel
                    # so all four score matmuls can row-pack concurrently
                    nc.vector.tensor_scalar(
                        ptb[:].bitcast(i16), s4b[:], FE_A, maskb[:, j : j + 1],
                        ALU.mult, ALU.add,
                    )
                    return pta, ptb

                def emit_pv(pta, ptb, j, c):
                    sj = (j == 0)
                    ej = (j == 7)
                    # fused PV+rowsum: stationary is V_h | ones [128, 33], so
                    # each head's matmul emits its O^T strip (rows 0-31 of the
                    # 33-row block) AND its softmax denominator (row 32) from a
                    # single moving stream. 33 output rows round up to a 64-col
                    # PE tile, so the two heads of a pair sit at tile columns 0
                    # and 64 (psum rows 0-32 / 64-96).
                    for hh in range(4):
                        h = 4 * grp + hh
                        ov = ov0 if hh < 2 else ov1
                        pt = pta if hh < 2 else ptb
                        vpos = 64 * (hh % 2)
                        nc.tensor.matmul(
                            ov[vpos : vpos + 33, 512 * c : 512 * (c + 1)],
                            va[:, j, h, :],
                            pt[:, 512 * (hh % 2) : 512 * (hh % 2 + 1)],
                            start=sj,
                            stop=ej,
                            tile_position=(0, vpos),
                        )

                st0 = stage_pool.tile([128, 1024], f32, tag="st", name=f"st0_{grp}")
                st1 = stage_pool.tile([128, 1024], f32, tag="st", name=f"st1_{grp}")
                se4 = norm_pool.tile([4, 1024], f32, tag="se4", name=f"se4_{grp}")
                rc4 = norm_pool.tile([4, 1024], bf16, tag="rc4", name=f"rc4_{grp}")

                def emit_half_tail(c):
                    # the c-major order finishes each 512-query half's PV
                    # accumulation 8 iterations before the grp ends: stage it
                    # and extract its denominators while the loop continues.
                    # With the fused PV+rowsum, denominators live at psum rows
                    # 32 (even head) and 96 (odd head) of each pair psum.
                    sl = slice(512 * c, 512 * (c + 1))
                    last = grp == 1 and c == 1
                    # staging split across ScalarE and DVE so neither exp
                    # stream takes the full 1.4us boundary insert
                    nc.scalar.copy(st0[:, sl], ov0[:, sl])
                    nc.vector.tensor_copy(st1[:, sl], ov1[:, sl])
                    nc.sync.dma_start(se4[0:1, sl], st0[32:33, sl])
                    nc.gpsimd.dma_start(se4[1:2, sl], st0[96:97, sl])
                    nc.sync.dma_start(se4[2:3, sl], st1[32:33, sl])
                    nc.scalar.dma_start(se4[3:4, sl], st1[96:97, sl])
                    if not last:
                        # reciprocal for this half on ScalarE (Ln + Exp share
                        # the loaded table set): ~1.3us riding the exp stream's
                        # slack, so the exit tail only handles the final half
                        lnh = norm_pool.tile([4, 512], f32, tag="ln4", name=f"ln_{grp}{c}")
                        nc.scalar.activation(lnh[:], se4[:, sl], AF.Ln)
                        nc.scalar.activation(rc4[:, sl], lnh[:], AF.Exp, scale=-1.0)

                iters = [(j, c) for c in range(2) for j in range(8)]
                pend = None
                for idx in range(len(iters) + 1):
                    if idx < len(iters):
                        j, c = iters[idx]
                        cur = (*emit_scores(j, c), j, c)
                        if grp == 0 and idx == 0:
                            # V projections and the remaining Q/K tiles fill the
                            # first exp latencies; emitted before any PV so the
                            # ov accumulators don't yet hold the psum slots
                            v_tile(0)
                            v_tile(1)
                            for jj in range(2, 6):
                                v_tile(jj)
                        if grp == 0 and idx == 1:
                            v_tile(6)
                            v_tile(7)
                            qk_tile(0, chunks=(1,))
                            qk_tile(2, chunks=(1,))
                            qk_tile(1)
                            qk_tile(3)
                    else:
                        cur = None
                    if pend is not None:
                        emit_pv(*pend)
                        if pend[2] == 7:
                            emit_half_tail(pend[3])
                    pend = cur

                # ---- last-half reciprocal (the other halves ran mid-loop) ----
                if grp == 1:
                    ln4 = norm_pool.tile([4, 512], f32, tag="ln4", name=f"ln4_{grp}")
                    nc.scalar.activation(ln4[:], se4[:, 512:1024], AF.Ln)
                    nc.scalar.activation(rc4[:, 512:1024], ln4[:], AF.Exp, scale=-1.0)
                saved[grp] = (st0, st1, rc4)

            # ---- deferred normalization: broadcast 1/denominator and scale ----
            # bca rows 0-31 = 1/d(h_even), rows 64-95 = 1/d(h_odd) (matches the
            # staged psum layout); bcb likewise for the st1 pair. The nx tiles
            # are then strip-assembled into otf's head-major channel order with
            # cross-partition SBUF->SBUF DMAs (DMA is the only engine that can
            # move data across partitions). grp1 is split per 512-query half so
            # the c0 chain (reciprocal precomputed mid-loop) drains while the
            # c1 chain waits on the post-loop Ln+Exp.
            bct = {}
            ntiles = {}
            for grp in range(2):
                st0, st1, rc4 = saved[grp]
                bca = ps_s4a.tile([128, 1024], f32, tag="s4a", name=f"bca{grp}")
                bcb = ps_s4b.tile([128, 1024], f32, tag="s4b", name=f"bcb{grp}")
                bct[grp] = (bca, bcb)
                ntiles[grp] = (
                    pt_pool.tile([128, 1024], bf16, tag="pt", name=f"n0_{grp}"),
                    pt_pool.tile([128, 1024], bf16, tag="pt", name=f"n1_{grp}"),
                )

            def emit_norm(grp, cs):
                st0, st1, rc4 = saved[grp]
                bca, bcb = bct[grp]
                n0, n1 = ntiles[grp]
                sl = slice(512 * cs[0], 512 * (cs[-1] + 1))
                for c in cs:
                    csl = slice(512 * c, 512 * (c + 1))
                    nc.tensor.matmul(
                        bca[:, csl], sel[:, 0:128], rc4[:, csl], start=True, stop=True
                    )
                    nc.tensor.matmul(
                        bcb[:, csl], sel[:, 128:256], rc4[:, csl], start=True, stop=True
                    )
                nc.vector.tensor_mul(n0[:, sl], st0[:, sl], bca[:, sl])
                nc.vector.tensor_mul(n1[:, sl], st1[:, sl], bcb[:, sl])
                gb = slice(1024 * grp + sl.start, 1024 * grp + sl.stop)
                # three parallel trigger queues so all four strips land ~together
                eng0 = nc.sync if grp == 0 else nc.scalar
                eng0.dma_start(otf[0:32, gb], n0[0:32, sl])
                nc.gpsimd.dma_start(otf[32:64, gb], n0[64:96, sl])
                nc.sync.dma_start(otf[64:96, gb], n1[0:32, sl])
                nc.gpsimd.dma_start(otf[96:128, gb], n1[64:96, sl])

            def emit_proj(c):
                for m in range(2):
                    ps = ps_ot.tile([128, 512], f32, tag="ot")
                    for t in range(2):
                        nc.tensor.matmul(
                            ps[:],
                            pw[:, 256 * t + 128 * m : 256 * t + 128 * (m + 1)],
                            otf[:, 1024 * t + 512 * c : 1024 * t + 512 * (c + 1)],
                            start=(t == 0),
                            stop=(t == 1),
                        )
                    ysl = slice(1024 * m + 512 * c, 1024 * m + 512 * (c + 1))
                    # proj bias is applied host-side; c1 copies ride the
                    # by-then-idle ScalarE so DVE isn't the tail bottleneck
                    (nc.vector.tensor_copy if c == 0 else nc.scalar.copy)(
                        ytile[:, ysl], ps[:]
                    )
                    (nc.sync if c == 0 else nc.gpsimd).dma_start(
                        yt_d[:, ysl], ytile[:, ysl]
                    )

            emit_norm(0, (0, 1))
            emit_norm(1, (0,))
            emit_norm(1, (1,))
            emit_proj(0)
            emit_proj(1)

    return nc


def _get_nc():
    if "nc" not in _cached:
        _cached["nc"] = _build_nc()
    return _cached["nc"]


def _pack_per_partition(a2d):
    """[2*128, F] -> [128, 2*F] with tile t at cols F*t."""
    n, f = a2d.shape
    t = n // 128
    return np.ascontiguousarray(
        a2d.reshape(t, 128, f).transpose(1, 0, 2).reshape(128, t * f)
    )


def _prepare(carrier_tokens, ct_mask, batch_num_windows, qkv_w, qkv_b, proj_w, proj_b):
    """Host-side bookkeeping: ragged->padded scatter, weight packing.
    Returns (in_maps, ctx) where ctx carries what postprocessing needs."""
    carrier_tokens = np.asarray(carrier_tokens, dtype=np.float32)
    ct_mask = np.asarray(ct_mask, dtype=np.float32)
    lens = np.asarray(batch_num_windows).astype(np.int64)
    qkv_w = np.asarray(qkv_w, dtype=np.float32)
    qkv_b = np.asarray(qkv_b, dtype=np.float32)
    proj_w = np.asarray(proj_w, dtype=np.float32)
    proj_b = np.asarray(proj_b, dtype=np.float32)

    total = carrier_tokens.shape[0]

    # ragged -> padded bookkeeping (mirrors the reference's scatter semantics:
    # OOB scatter indices dropped, OOB gather indices clipped)
    offsets = np.concatenate([[0], np.cumsum(lens)])
    tok = np.arange(total)
    b_id = np.searchsorted(offsets[1:], tok, side="right")
    w_id = tok - offsets[np.minimum(b_id, B)]
    flat_idx = b_id * MAXW + w_id
    valid = flat_idx < B * MAXW
    padded = np.zeros((B * MAXW, C), np.float32)
    padded[flat_idx[valid]] = carrier_tokens[valid]
    padded = padded.reshape(B, MAXW, C)

    mask_col = np.ascontiguousarray(ct_mask[:, 0, :])  # [B, MAXW]

    # host-side exact weight transforms; the effective proj bias (V bias
    # folded in via softmax-sums-to-1) is applied host-side in _postprocess
    pb_eff = qkv_b[2 * C : 3 * C] @ proj_w + proj_b

    qw_packed = _pack_per_partition(qkv_w)                      # [128, 1536]
    qb_packed = np.ascontiguousarray(qkv_b[0:C].reshape(2, 128).T)
    pw_packed = _pack_per_partition(proj_w)                     # [128, 512]

    import ml_dtypes
    # sel[:, 0:128] broadcasts 1/d of the even/odd head of the st0 pair to
    # rows 0-31 / 64-95; sel[:, 128:256] likewise for the st1 pair
    sel_arr = np.zeros((4, 256), ml_dtypes.bfloat16)
    sel_arr[0, 0:32] = 1.0
    sel_arr[1, 64:96] = 1.0
    sel_arr[2, 128:160] = 1.0
    sel_arr[3, 192:224] = 1.0
    qw_packed = qw_packed.astype(ml_dtypes.bfloat16)
    pw_packed = pw_packed.astype(ml_dtypes.bfloat16)
    # fast-exp per-partition bias: maskb = mask*log2(e)*2^7 + (2^7*127 - c)
    # with the calibrated correction c = 11.5 (int16 round-to-nearest)
    LOG2E = 1.4426950408889634
    FE_B0 = np.float32(16256.0 - 11.5)
    in_maps = []
    for b in range(B):
        xt = _pack_per_partition(padded[b].T).astype(ml_dtypes.bfloat16)
        mb = np.ascontiguousarray(mask_col[b].reshape(8, 128).T)
        mbb = (mb * np.float32(LOG2E * 128.0) + FE_B0).astype(np.float32)
        in_maps.append(
            {
                "xt": xt,
                "qw": qw_packed,
                "qb": qb_packed,
                "pw": pw_packed,
                "mask": mb,
                "maskb": mbb,
                "sel": sel_arr,
            }
        )

    ctx = {
        "flat_idx": flat_idx,
        "mask_col": mask_col,
        "padded": padded,
        "qkv_w": qkv_w,
        "qkv_b": qkv_b,
        "proj_w": proj_w,
        "proj_b": proj_b,
        "pb_eff": pb_eff,
    }
    return in_maps, ctx


def _postprocess(results, ctx):
    """Per-core outputs -> full ragged output (gather + degenerate-row fix)."""
    flat_idx = ctx["flat_idx"]
    mask_col = ctx["mask_col"]
    padded = ctx["padded"]
    qkv_w, qkv_b = ctx["qkv_w"], ctx["qkv_b"]
    proj_w, proj_b = ctx["proj_w"], ctx["proj_b"]

    y_pad = np.empty((B, MAXW, C), np.float32)
    for b in range(B):
        yt = np.asarray(results[b]["yt"], dtype=np.float32)     # [128, 2048]
        y_t = yt.reshape(128, 2, MAXW).transpose(1, 0, 2).reshape(C, MAXW)
        y_pad[b] = y_t.T
    y_flat = y_pad.reshape(B * MAXW, C)
    gather_idx = np.clip(flat_idx, 0, B * MAXW - 1)
    # the effective proj bias is applied here (host side) rather than on-chip
    out = y_flat[gather_idx] + ctx["pb_eff"].astype(np.float32)[None, :]

    # degenerate rows: gathered positions whose key mask is fully masked.
    # The reference's softmax (with max-subtraction) gives uniform weights
    # there; our exp underflows to 0/0. Recompute those rows exactly.
    row_b = np.minimum(gather_idx // MAXW, B - 1)
    degenerate_batches = [b for b in range(B) if np.all(mask_col[b] < NEG_THRESH)]
    for b in degenerate_batches:
        rows = np.nonzero(row_b == b)[0]
        if rows.size == 0:
            continue
        vmat = padded[b] @ qkv_w[:, 2 * C : 3 * C] + qkv_b[2 * C : 3 * C]
        mean_v = vmat.mean(axis=0)  # uniform attention, same for all heads
        fix = mean_v @ proj_w + proj_b
        out[rows] = fix.astype(np.float32)

    return np.ascontiguousarray(out.astype(np.float32))


def run_device(in_maps, **spmd_kwargs):
    from concourse import bass_utils

    nc = _get_nc()
    return bass_utils.run_bass_kernel_spmd(
        nc, in_maps, core_ids=list(range(B)), **spmd_kwargs
    )


def kernel(carrier_tokens, ct_mask, batch_num_windows, qkv_w, qkv_b, proj_w, proj_b):
    in_maps, ctx = _prepare(
        carrier_tokens, ct_mask, batch_num_windows, qkv_w, qkv_b, proj_w, proj_b
    )
    res = run_device(in_maps, trace=False)
    return _postprocess(res.results, ctx)



# revision 51
# speedup vs baseline: 1.0142x; 1.0138x over previous
"""CTAttention Trainium2 kernel.

Full-input contract: kernel(**inputs) takes the unsharded numpy inputs and
returns the full [total, C] output. Internally: data-parallel over the batch
axis B=8 across 8 NeuronCores (attention is independent per batch element);
qkv/proj weights replicated; ragged scatter/gather bookkeeping on the host.

Per-core dataflow (batch b, dense 1024 windows, 8 heads, head_dim 32):
  X^T[256,1024] -> Q^T/K^T (bf16, channel-on-partition) and V[kpos,ch] (bf16)
  attention runs 4 heads (one group) at a time, software-pipelined, with
  c-major (query-chunk-major) iteration so each 512-query half's PV
  accumulation finishes early and its staging/denominator extraction hides
  under the remaining loop:
    scores: S^T = per-head K=32 matmuls, row-packed on the PE array (bf16,
            one 32-row strip per head), into two [128,1024] psums
    exp:    heads 0,1 of the group: ScalarE Exp with the key-padding mask as
            a per-partition bias (masked scores underflow to exactly 0, so no
            row-max pass is needed). heads 2,3: Schraudolph fast-exp on the
            DVE — one tensor_scalar (s*A + maskb) with int16 round-to-nearest
            output whose bits ARE the bf16 P (int16 saturation maps masked
            scores to 0x8000 = -0.0). Splitting whole heads (not key blocks)
            keeps each query's softmax internally consistent, so the ~3%
            fast-exp wiggle largely cancels through the normalization.
    PV:     fused PV+rowsum: stationary is V_h|ones [128,33], so each head's
            matmul emits its O^T strip AND its softmax denominator (psum row
            32/96) from a single moving stream; the two heads of a pair sit
            at PE column tiles 0 and 64.
  normalization: denominators -> 1/x via ScalarE Ln+Exp per 512-query half
  (same ACT table set as Exp/Copy: no table switches), three of four halves
  hidden under the loop; broadcast via K=4 selector matmuls, one DVE multiply
  per pair, then cross-partition strip assembly into head-major channel order
  with SBUF->SBUF DMAs. Output projection consumes the assembled O^T; the
  proj bias is applied host-side; y is returned in bf16.

Exact algebraic simplifications vs the reference:
  - K bias dropped (softmax is invariant to per-query constant shifts)
  - V bias folded into the proj bias (softmax weights sum to 1, applied host-side)
  - head-dim scale folded into the exp's input scale / fast-exp multiplier

Environment workarounds (this walrus build): at most one sem wait per
instruction (waits hoisted onto injected NOPs), fp32/fp32r matmuls require
dst partition base 0, no gpsimd extended instructions (and GpSimd cannot
access PSUM), DMA cannot access PSUM or broadcast (zero-stride APs rejected).
"""

import sys

if "/opt/trn_rl_repo" not in sys.path:
    sys.path.insert(0, "/opt/trn_rl_repo")

import numpy as np

B = 8
C = 256
H = 8
HD = 32
MAXW = 1024
SCALE = HD ** -0.5
NEG_THRESH = -1e8  # mask values below this count as fully masked

_cached = {}


def _build_nc():
    import bass_rust
    import concourse.bass as bass
    import concourse.tile as tile
    import concourse.mybir as mybir
    from concourse.vector_clock import ScopedClock

    # ---- workaround: this walrus build accepts at most ONE sem wait per
    # instruction ("Too many sync wait commands" in setupSyncWait). Tile
    # attaches multi-sem waits freely. Split: hoist all but the last wait of
    # every committed instruction onto injected same-engine NOPs, and split
    # the final drain the same way.
    _ctr = [0]

    def _hoist_excess_waits(tc_self, inst, orig_add):
        si = inst.sync_info
        if si is not None:
            waits = list(si.on_wait or [])
            if len(waits) > 1:
                for w in waits[:-1]:
                    _ctr[0] += 1
                    nop = mybir.InstNoOp(name=f"waitsplit-{_ctr[0]}")
                    nop.engine = inst.engine
                    nop.sync_info = bass_rust.SyncInfo(on_wait=[w], on_update=[])
                    orig_add(tc_self, nop)
                si.on_wait = waits[-1:]
        orig_add(tc_self, inst)

    if not getattr(tile.TileContext, "_waitsplit_patched", False):
        _orig_add_instruction = tile.TileContext._add_instruction

        def _split_add_instruction(self, inst):
            _hoist_excess_waits(self, inst, _orig_add_instruction)

        tile.TileContext._add_instruction = _split_add_instruction

        def _patched_drain_and_barrier(self, tick_clock, wait_clock):
            # Trimmed exit: drain + one barrier. Semaphore clears and the
            # second barrier are dropped — each NEFF execution reinitializes
            # semaphores at program start, so the ~6us of teardown they cost
            # buys nothing (re-execution verified against the first run).
            nc = self.nc
            d0 = nc.sync.drain()
            wait_clock.add_sem_waits(
                d0.ins, ScopedClock({None: tick_clock.global_clock})
            )
            si = d0.ins.sync_info
            waits = list(si.on_wait) if si is not None else []
            if len(waits) > 1:
                si.on_wait = waits[0:1]
                for w in waits[1:]:
                    dk = nc.sync.drain()
                    dk.ins.sync_info = bass_rust.SyncInfo(on_wait=[w], on_update=[])
            nc.all_engine_barrier()
            assert self.sems is not None
            popped = nc._tile_sem_poison_stack.pop()
            assert popped is self._sem_poison

        tile.TileContext._drain_and_barrier = _patched_drain_and_barrier
        tile.TileContext._waitsplit_patched = True

    dt = mybir.dt
    f32 = dt.float32
    f32r = dt.float32r
    i16 = dt.int16
    AF = mybir.ActivationFunctionType
    ALU = mybir.AluOpType

    nc = bass.Bass(
        "TRN2",
        target_bir_lowering=False,
        debug=False,
        num_devices=1,
        enable_asserts=False,
    )

    bf16 = dt.bfloat16

    xt_d = nc.dram_tensor("xt", [128, 2048], bf16, kind="ExternalInput").ap()
    qw_d = nc.dram_tensor("qw", [128, 1536], bf16, kind="ExternalInput").ap()
    qb_d = nc.dram_tensor("qb", [128, 2], f32, kind="ExternalInput").ap()
    pw_d = nc.dram_tensor("pw", [128, 512], bf16, kind="ExternalInput").ap()
    mask_d = nc.dram_tensor("mask", [128, 8], f32, kind="ExternalInput").ap()
    maskb_d = nc.dram_tensor("maskb", [128, 8], f32, kind="ExternalInput").ap()
    sel_d = nc.dram_tensor("sel", [4, 256], bf16, kind="ExternalInput").ap()
    yt_d = nc.dram_tensor("yt", [128, 2048], bf16, kind="ExternalOutput").ap()

    # Schraudolph fast-exp (DVE): bf16(bits) = int16(round(s*FE_A + maskb))
    # where FE_A = SCALE*log2(e)*2^7; int16 saturation maps masked scores to
    # 0x8000 = -0.0. Calibrated bias constant folded into maskb host-side.
    FE_A = float(SCALE * 1.4426950408889634 * 128.0)

    with tile.TileContext(nc) as tc:
        with (
            tc.tile_pool(name="const", bufs=1) as const_pool,
            tc.tile_pool(name="big", bufs=1) as big_pool,
            tc.tile_pool(name="pt", bufs=6) as pt_pool,
            tc.tile_pool(name="stage", bufs=4) as stage_pool,
            tc.tile_pool(name="norm", bufs=2) as norm_pool,
            tc.tile_pool(name="ps_s4a", bufs=1, space="PSUM") as ps_s4a,
            tc.tile_pool(name="ps_s4b", bufs=1, space="PSUM") as ps_s4b,
            tc.tile_pool(name="ps_ot", bufs=2, space="PSUM") as ps_ot,
        ):
            xt = const_pool.tile([128, 2048], bf16, tag="xt")
            qw = const_pool.tile([128, 1536], bf16, tag="qw")
            qb = const_pool.tile([128, 2], f32, tag="qb")
            pw = const_pool.tile([128, 512], bf16, tag="pw")
            mask = const_pool.tile([128, 8], f32, tag="mask")
            maskb = const_pool.tile([128, 8], f32, tag="maskb")
            sel = const_pool.tile([4, 256], bf16, tag="sel")

            # startup DMAs ordered by first consumer: the first qk tiles need
            # xt chunks 0/2 and qw cols [0:384]+[768:1152]; mask gates the
            # first exp. sync/scalar/gpsimd queues run in parallel.
            nc.gpsimd.dma_start(mask[:], mask_d)
            nc.gpsimd.dma_start(maskb[:], maskb_d)
            # qw slices in exact first-consumer order: qk0 needs [0:128]+
            # [768:896], qk2 needs [256:384]+[1024:1152]; tiny first transfers
            # shave the DMA-latency-bound startup chain
            nc.sync.dma_start(xt[:, 0:512], xt_d[:, 0:512])
            nc.scalar.dma_start(qw[:, 0:128], qw_d[:, 0:128])
            nc.sync.dma_start(xt[:, 1024:1536], xt_d[:, 1024:1536])
            nc.scalar.dma_start(qw[:, 768:896], qw_d[:, 768:896])
            nc.scalar.dma_start(qw[:, 256:384], qw_d[:, 256:384])
            nc.scalar.dma_start(qw[:, 1024:1152], qw_d[:, 1024:1152])
            nc.sync.dma_start(xt[:, 512:1024], xt_d[:, 512:1024])
            nc.scalar.dma_start(qw[:, 128:256], qw_d[:, 128:256])
            nc.scalar.dma_start(qw[:, 896:1024], qw_d[:, 896:1024])
            nc.sync.dma_start(xt[:, 1536:2048], xt_d[:, 1536:2048])
            nc.scalar.dma_start(qw[:, 384:768], qw_d[:, 384:768])
            nc.scalar.dma_start(qw[:, 1152:1536], qw_d[:, 1152:1536])
            # dependency-free ACT table warmup (forces the Exp table load now,
            # after the critical qw DMA triggers so it doesn't delay them)
            warm_in = const_pool.tile([1, 2], f32, tag="warm_in")
            nc.vector.memset(warm_in[:], 0.0)
            warm = const_pool.tile([1, 2], f32, tag="warm")
            nc.scalar.activation(warm[:], warm_in[:], AF.Exp, scale=0.0)
            nc.gpsimd.dma_start(qb[:], qb_d)
            nc.gpsimd.dma_start(sel[:], sel_d)
            nc.gpsimd.dma_start(pw[:], pw_d)

            qt = big_pool.tile([128, 2048], bf16, tag="qt")
            kt = big_pool.tile([128, 2048], bf16, tag="kt")
            # [part, kpos_blk, head, head_dim+1]; col 32 = the all-ones column
            # that makes each PV matmul emit the softmax denominator as an
            # extra psum row (stationary = V_h | ones -> out rows 0-31 = O^T
            # strip, row 32 = rowsum)
            va = big_pool.tile([128, 8, 8, 33], bf16, tag="va")
            otf = big_pool.tile([128, 2048], bf16, tag="otf")
            ytile = big_pool.tile([128, 2048], bf16, tag="ytile")
            nc.vector.memset(va[:, :, :, 32:33], 1.0)

            # ---------- qkv projections ----------
            # V first (PV needs all of it), then the Q/K tiles needed by the
            # first head group; the rest is emitted between the groups so the
            # PE fills ACT-bound gaps.
            def qk_tile(m, chunks=(0, 1), pool=None, ptag="ot"):
                pool = pool if pool is not None else ps_ot
                for c in chunks:
                    ps = pool.tile([128, 512], f32, tag=ptag, name=f"qk{m}{c}")
                    for t in range(2):
                        nc.tensor.matmul(
                            ps[:],
                            qw[:, 768 * t + 128 * m : 768 * t + 128 * (m + 1)],
                            xt[:, 1024 * t + 512 * c : 1024 * t + 512 * (c + 1)],
                            start=(t == 0),
                            stop=(t == 1),
                        )
                    if m < 2:
                        nc.vector.tensor_scalar_add(
                            qt[:, 1024 * m + 512 * c : 1024 * m + 512 * (c + 1)],
                            ps[:],
                            qb[:, m : m + 1],
                        )
                    else:
                        nc.vector.tensor_copy(
                            kt[:, 1024 * (m - 2) + 512 * c : 1024 * (m - 2) + 512 * (c + 1)],
                            ps[:],
                        )

            # V: out[kpos_block, cv] in bf16 (no bias; folded into proj bias)
            def v_tile(j, pool=None, ptag="ot"):
                pool = pool if pool is not None else ps_ot
                ps = pool.tile([128, 256], f32, tag=ptag, name=f"v{j}")
                for t in range(2):
                    nc.tensor.matmul(
                        ps[:],
                        xt[:, 1024 * t + 128 * j : 1024 * t + 128 * (j + 1)],
                        qw[:, 768 * t + 512 : 768 * t + 768],
                        start=(t == 0),
                        stop=(t == 1),
                    )
                nc.vector.tensor_copy(
                    va[:, j, :, 0:32],
                    ps[:].rearrange("p (h d) -> p h d", d=32),
                )

            # only the chunk-0 Q/K tiles gate the first score matmul; the rest
            # of the projections are emitted inside the first iterations
            qk_tile(0, chunks=(0,))
            qk_tile(2, chunks=(0,))

            # ---------- attention: 4 heads (one group) at a time ----------
            # scores: 4-way row-packed fp32r matmuls, two [128,1024] psum tiles
            # per step (double-buffered so ScalarE exps run back-to-back), one
            # exp per tile -> bf16 P^T, then 8 bf16 PV/rowsum matmuls col-tiled
            # across the 4 PE column strips.
            # O^T strips: head hh at psum rows 32*hh of its pair psum
            # (pair 0 = hh 0,1 rows 0-63; pair 1 = hh 2,3 rows 64-127);
            # denominator rows: hh -> (64, 96, 0, 32).
            saved = {}
            for grp in range(2):
                ov0 = ps_ot.tile([128, 1024], f32, tag="ot", name=f"ov0_{grp}")
                ov1 = ps_ot.tile([128, 1024], f32, tag="ot", name=f"ov1_{grp}")

                def emit_scores(j, c):
                    s4a = ps_s4a.tile([128, 1024], f32, tag="s4a", name=f"s4a{grp}{j}{c}")
                    s4b = ps_s4b.tile([128, 1024], f32, tag="s4b", name=f"s4b{grp}{j}{c}")
                    for hh in range(4):
                        s4 = s4a if hh < 2 else s4b
                        base = 32 * hh
                        nc.tensor.matmul(
                            s4[:, 512 * (hh % 2) : 512 * (hh % 2 + 1)],
                            kt[base : base + 32,
                               1024 * grp + 128 * j : 1024 * grp + 128 * (j + 1)],
                            qt[base : base + 32,
                               1024 * grp + 512 * c : 1024 * grp + 512 * (c + 1)],
                            start=True,
                            stop=True,
                            tile_position=(base, 0),
                        )
                    pta = pt_pool.tile([128, 1024], bf16, tag="pt", name=f"pta{grp}{j}{c}")
                    ptb = pt_pool.tile([128, 1024], bf16, tag="pt", name=f"ptb{grp}{j}{c}")
                    nc.scalar.activation(
                        pta[:], s4a[:], AF.Exp, bias=mask[:, j : j + 1], scale=SCALE,
                    )
                    # heads 2,3 of the group take the DVE fast-exp: halves the
                    # ScalarE stream and releases both score psums in parallel
                    # so all four score matmuls can row-pack concurrently
                    nc.vector.tensor_scalar(
                        ptb[:].bitcast(i16), s4b[:], FE_A, maskb[:, j : j + 1],
                        ALU.mult, ALU.add,
                    )
                    return pta, ptb

                def emit_pv(pta, ptb, j, c):
                    sj = (j == 0)
                    ej = (j == 7)
                    # fused PV+rowsum: stationary is V_h | ones [128, 33], so
                    # each head's matmul emits its O^T strip (rows 0-31 of the
                    # 33-row block) AND its softmax denominator (row 32) from a
                    # single moving stream. 33 output rows round up to a 64-col
                    # PE tile, so the two heads of a pair sit at tile columns 0
                    # and 64 (psum rows 0-32 / 64-96).
                    for hh in range(4):
                        h = 4 * grp + hh
                        ov = ov0 if hh < 2 else ov1
                        pt = pta if hh < 2 else ptb
                        vpos = 64 * (hh % 2)
                        nc.tensor.matmul(
                            ov[vpos : vpos + 33, 512 * c : 512 * (c + 1)],
                            va[:, j, h, :],
                            pt[:, 512 * (hh % 2) : 512 * (hh % 2 + 1)],
                            start=sj,
                            stop=ej,
                            tile_position=(0, vpos),
                        )

                st0 = stage_pool.tile([128, 1024], f32, tag="st", name=f"st0_{grp}")
                st1 = stage_pool.tile([128, 1024], f32, tag="st", name=f"st1_{grp}")
                se4 = norm_pool.tile([4, 1024], f32, tag="se4", name=f"se4_{grp}")
                rc4 = norm_pool.tile([4, 1024], bf16, tag="rc4", name=f"rc4_{grp}")

                def emit_half_tail(c):
                    # the c-major order finishes each 512-query half's PV
                    # accumulation 8 iterations before the grp ends: stage it
                    # and extract its denominators while the loop continues.
                    # With the fused PV+rowsum, denominators live at psum rows
                    # 32 (even head) and 96 (odd head) of each pair psum.
                    sl = slice(512 * c, 512 * (c + 1))
                    last = grp == 1 and c == 1
                    # staging split across ScalarE and DVE so neither exp
                    # stream takes the full 1.4us boundary insert
                    nc.scalar.copy(st0[:, sl], ov0[:, sl])
                    nc.vector.tensor_copy(st1[:, sl], ov1[:, sl])
                    nc.sync.dma_start(se4[0:1, sl], st0[32:33, sl])
                    nc.gpsimd.dma_start(se4[1:2, sl], st0[96:97, sl])
                    nc.sync.dma_start(se4[2:3, sl], st1[32:33, sl])
                    nc.scalar.dma_start(se4[3:4, sl], st1[96:97, sl])
                    if not last:
                        # reciprocal for this half on ScalarE (Ln + Exp share
                        # the loaded table set): ~1.3us riding the exp stream's
                        # slack, so the exit tail only handles the final half
                        lnh = norm_pool.tile([4, 512], f32, tag="ln4", name=f"ln_{grp}{c}")
                        nc.scalar.activation(lnh[:], se4[:, sl], AF.Ln)
                        nc.scalar.activation(rc4[:, sl], lnh[:], AF.Exp, scale=-1.0)

                iters = [(j, c) for c in range(2) for j in range(8)]
                pend = None
                for idx in range(len(iters) + 1):
                    if idx < len(iters):
                        j, c = iters[idx]
                        cur = (*emit_scores(j, c), j, c)
                        if grp == 0 and idx == 0:
                            # V projections and the remaining Q/K tiles fill the
                            # first exp latencies; emitted before any PV so the
                            # ov accumulators don't yet hold the psum slots
                            v_tile(0)
                            v_tile(1)
                            for jj in range(2, 6):
                                v_tile(jj)
                        if grp == 0 and idx == 1:
                            v_tile(6)
                            v_tile(7)
                            qk_tile(0, chunks=(1,))
                            qk_tile(2, chunks=(1,))
                            qk_tile(1)
                            qk_tile(3)
                    else:
                        cur = None
                    if pend is not None:
                        emit_pv(*pend)
                        if pend[2] == 7:
                            emit_half_tail(pend[3])
                    pend = cur

                # ---- last-half reciprocal (the other halves ran mid-loop) ----
                if grp == 1:
                    ln4 = norm_pool.tile([4, 512], f32, tag="ln4", name=f"ln4_{grp}")
                    nc.scalar.activation(ln4[:], se4[:, 512:1024], AF.Ln)
                    nc.scalar.activation(rc4[:, 512:1024], ln4[:], AF.Exp, scale=-1.0)
                saved[grp] = (st0, st1, rc4)

            # ---- deferred normalization: broadcast 1/denominator and scale ----
            # bca rows 0-31 = 1/d(h_even), rows 64-95 = 1/d(h_odd) (matches the
            # staged psum layout); bcb likewise for the st1 pair. The nx tiles
            # are then strip-assembled into otf's head-major channel order with
            # cross-partition SBUF->SBUF DMAs (DMA is the only engine that can
            # move data across partitions). grp1 is split per 512-query half so
            # the c0 chain (reciprocal precomputed mid-loop) drains while the
            # c1 chain waits on the post-loop Ln+Exp.
            bct = {}
            ntiles = {}
            for grp in range(2):
                st0, st1, rc4 = saved[grp]
                bca = ps_s4a.tile([128, 1024], f32, tag="s4a", name=f"bca{grp}")
                bcb = ps_s4b.tile([128, 1024], f32, tag="s4b", name=f"bcb{grp}")
                bct[grp] = (bca, bcb)
                ntiles[grp] = (
                    pt_pool.tile([128, 1024], bf16, tag="pt", name=f"n0_{grp}"),
                    pt_pool.tile([128, 1024], bf16, tag="pt", name=f"n1_{grp}"),
                )

            def emit_norm(grp, cs):
                st0, st1, rc4 = saved[grp]
                bca, bcb = bct[grp]
                n0, n1 = ntiles[grp]
                sl = slice(512 * cs[0], 512 * (cs[-1] + 1))
                for c in cs:
                    csl = slice(512 * c, 512 * (c + 1))
                    nc.tensor.matmul(
                        bca[:, csl], sel[:, 0:128], rc4[:, csl], start=True, stop=True
                    )
                    nc.tensor.matmul(
                        bcb[:, csl], sel[:, 128:256], rc4[:, csl], start=True, stop=True
                    )
                nc.vector.tensor_mul(n0[:, sl], st0[:, sl], bca[:, sl])
                nc.vector.tensor_mul(n1[:, sl], st1[:, sl], bcb[:, sl])
                gb = slice(1024 * grp + sl.start, 1024 * grp + sl.stop)
                # three parallel trigger queues so all four strips land ~together
                eng0 = nc.sync if grp == 0 else nc.scalar
                eng0.dma_start(otf[0:32, gb], n0[0:32, sl])
                nc.gpsimd.dma_start(otf[32:64, gb], n0[64:96, sl])
                nc.sync.dma_start(otf[64:96, gb], n1[0:32, sl])
                nc.gpsimd.dma_start(otf[96:128, gb], n1[64:96, sl])

            def emit_proj(c):
                for m in range(2):
                    ps = ps_ot.tile([128, 512], f32, tag="ot")
                    for t in range(2):
                        nc.tensor.matmul(
                            ps[:],
                            pw[:, 256 * t + 128 * m : 256 * t + 128 * (m + 1)],
                            otf[:, 1024 * t + 512 * c : 1024 * t + 512 * (c + 1)],
                            start=(t == 0),
                            stop=(t == 1),
                        )
                    ysl = slice(1024 * m + 512 * c, 1024 * m + 512 * (c + 1))
                    # proj bias is applied host-side; c1 copies ride the
                    # by-then-idle ScalarE so DVE isn't the tail bottleneck
                    (nc.vector.tensor_copy if c == 0 else nc.scalar.copy)(
                        ytile[:, ysl], ps[:]
                    )
                    (nc.sync if c == 0 else nc.gpsimd).dma_start(
                        yt_d[:, ysl], ytile[:, ysl]
                    )

            emit_norm(0, (0, 1))
            emit_norm(1, (0,))
            emit_norm(1, (1,))
            emit_proj(0)
            emit_proj(1)

    return nc


def _get_nc():
    if "nc" not in _cached:
        _cached["nc"] = _build_nc()
    return _cached["nc"]


def _pack_per_partition(a2d):
    """[2*128, F] -> [128, 2*F] with tile t at cols F*t."""
    n, f = a2d.shape
    t = n // 128
    return np.ascontiguousarray(
        a2d.reshape(t, 128, f).transpose(1, 0, 2).reshape(128, t * f)
    )


def _prepare(carrier_tokens, ct_mask, batch_num_windows, qkv_w, qkv_b, proj_w, proj_b):
    """Host-side bookkeeping: ragged->padded scatter, weight packing.
    Returns (in_maps, ctx) where ctx carries what postprocessing needs."""
    carrier_tokens = np.asarray(carrier_tokens, dtype=np.float32)
    ct_mask = np.asarray(ct_mask, dtype=np.float32)
    lens = np.asarray(batch_num_windows).astype(np.int64)
    qkv_w = np.asarray(qkv_w, dtype=np.float32)
    qkv_b = np.asarray(qkv_b, dtype=np.float32)
    proj_w = np.asarray(proj_w, dtype=np.float32)
    proj_b = np.asarray(proj_b, dtype=np.float32)

    total = carrier_tokens.shape[0]

    # ragged -> padded bookkeeping (mirrors the reference's scatter semantics:
    # OOB scatter indices dropped, OOB gather indices clipped)
    offsets = np.concatenate([[0], np.cumsum(lens)])
    tok = np.arange(total)
    b_id = np.searchsorted(offsets[1:], tok, side="right")
    w_id = tok - offsets[np.minimum(b_id, B)]
    flat_idx = b_id * MAXW + w_id
    valid = flat_idx < B * MAXW
    padded = np.zeros((B * MAXW, C), np.float32)
    padded[flat_idx[valid]] = carrier_tokens[valid]
    padded = padded.reshape(B, MAXW, C)

    mask_col = np.ascontiguousarray(ct_mask[:, 0, :])  # [B, MAXW]

    # host-side exact weight transforms; the effective proj bias (V bias
    # folded in via softmax-sums-to-1) is applied host-side in _postprocess
    pb_eff = qkv_b[2 * C : 3 * C] @ proj_w + proj_b

    qw_packed = _pack_per_partition(qkv_w)                      # [128, 1536]
    qb_packed = np.ascontiguousarray(qkv_b[0:C].reshape(2, 128).T)
    pw_packed = _pack_per_partition(proj_w)                     # [128, 512]

    import ml_dtypes
    # sel[:, 0:128] broadcasts 1/d of the even/odd head of the st0 pair to
    # rows 0-31 / 64-95; sel[:, 128:256] likewise for the st1 pair
    sel_arr = np.zeros((4, 256), ml_dtypes.bfloat16)
    sel_arr[0, 0:32] = 1.0
    sel_arr[1, 64:96] = 1.0
    sel_arr[2, 128:160] = 1.0
    sel_arr[3, 192:224] = 1.0
    qw_packed = qw_packed.astype(ml_dtypes.bfloat16)
    pw_packed = pw_packed.astype(ml_dtypes.bfloat16)
    # fast-exp per-partition bias: maskb = mask*log2(e)*2^7 + (2^7*127 - c)
    # with the calibrated correction c = 11.5 (int16 round-to-nearest)
    LOG2E = 1.4426950408889634
    FE_B0 = np.float32(16256.0 - 11.5)
    in_maps = []
    for b in range(B):
        xt = _pack_per_partition(padded[b].T).astype(ml_dtypes.bfloat16)
        mb = np.ascontiguousarray(mask_col[b].reshape(8, 128).T)
        mbb = (mb * np.float32(LOG2E * 128.0) + FE_B0).astype(np.float32)
        in_maps.append(
            {
                "xt": xt,
                "qw": qw_packed,
                "qb": qb_packed,
                "pw": pw_packed,
                "mask": mb,
                "maskb": mbb,
                "sel": sel_arr,
            }
        )

    ctx = {
        "flat_idx": flat_idx,
        "mask_col": mask_col,
        "padded": padded,
        "qkv_w": qkv_w,
        "qkv_b": qkv_b,
        "proj_w": proj_w,
        "proj_b": proj_b,
        "pb_eff": pb_eff,
    }
    return in_maps, ctx


def _postprocess(results, ctx):
    """Per-core outputs -> full ragged output (gather + degenerate-row fix)."""
    flat_idx = ctx["flat_idx"]
    mask_col = ctx["mask_col"]
    padded = ctx["padded"]
    qkv_w, qkv_b = ctx["qkv_w"], ctx["qkv_b"]
    proj_w, proj_b = ctx["proj_w"], ctx["proj_b"]

    y_pad = np.empty((B, MAXW, C), np.float32)
    for b in range(B):
        yt = np.asarray(results[b]["yt"], dtype=np.float32)     # [128, 2048]
        y_t = yt.reshape(128, 2, MAXW).transpose(1, 0, 2).reshape(C, MAXW)
        y_pad[b] = y_t.T
    y_flat = y_pad.reshape(B * MAXW, C)
    gather_idx = np.clip(flat_idx, 0, B * MAXW - 1)
    # the effective proj bias is applied here (host side) rather than on-chip
    out = y_flat[gather_idx] + ctx["pb_eff"].astype(np.float32)[None, :]

    # degenerate rows: gathered positions whose key mask is fully masked.
    # The reference's softmax (with max-subtraction) gives uniform weights
    # there; our exp underflows to 0/0. Recompute those rows exactly.
    row_b = np.minimum(gather_idx // MAXW, B - 1)
    degenerate_batches = [b for b in range(B) if np.all(mask_col[b] < NEG_THRESH)]
    for b in degenerate_batches:
        rows = np.nonzero(row_b == b)[0]
        if rows.size == 0:
            continue
        vmat = padded[b] @ qkv_w[:, 2 * C : 3 * C] + qkv_b[2 * C : 3 * C]
        mean_v = vmat.mean(axis=0)  # uniform attention, same for all heads
        fix = mean_v @ proj_w + proj_b
        out[rows] = fix.astype(np.float32)

    return np.ascontiguousarray(out.astype(np.float32))


def run_device(in_maps, **spmd_kwargs):
    from concourse import bass_utils

    nc = _get_nc()
    return bass_utils.run_bass_kernel_spmd(
        nc, in_maps, core_ids=list(range(B)), **spmd_kwargs
    )


def kernel(carrier_tokens, ct_mask, batch_num_windows, qkv_w, qkv_b, proj_w, proj_b):
    in_maps, ctx = _prepare(
        carrier_tokens, ct_mask, batch_num_windows, qkv_w, qkv_b, proj_w, proj_b
    )
    res = run_device(in_maps, trace=False)
    return _postprocess(res.results, ctx)



# revision 52
# speedup vs baseline: 1.0191x; 1.0048x over previous
"""CTAttention Trainium2 kernel.

Full-input contract: kernel(**inputs) takes the unsharded numpy inputs and
returns the full [total, C] output. Internally: data-parallel over the batch
axis B=8 across 8 NeuronCores (attention is independent per batch element);
qkv/proj weights replicated; ragged scatter/gather bookkeeping on the host.

Per-core dataflow (batch b, dense 1024 windows, 8 heads, head_dim 32):
  X^T[256,1024] -> Q^T/K^T (bf16, channel-on-partition) and V[kpos,ch] (bf16)
  attention runs 4 heads (one group) at a time, software-pipelined, with
  c-major (query-chunk-major) iteration so each 512-query half's PV
  accumulation finishes early and its staging/denominator extraction hides
  under the remaining loop:
    scores: S^T = per-head K=32 matmuls, row-packed on the PE array (bf16,
            one 32-row strip per head), into two [128,1024] psums
    exp:    heads 0,1 of the group: ScalarE Exp with the key-padding mask as
            a per-partition bias (masked scores underflow to exactly 0, so no
            row-max pass is needed). heads 2,3: Schraudolph fast-exp on the
            DVE — one tensor_scalar (s*A + maskb) with int16 round-to-nearest
            output whose bits ARE the bf16 P (int16 saturation maps masked
            scores to 0x8000 = -0.0). Splitting whole heads (not key blocks)
            keeps each query's softmax internally consistent, so the ~3%
            fast-exp wiggle largely cancels through the normalization.
    PV:     fused PV+rowsum: stationary is V_h|ones [128,33], so each head's
            matmul emits its O^T strip AND its softmax denominator (psum row
            32/96) from a single moving stream; the two heads of a pair sit
            at PE column tiles 0 and 64.
  normalization: denominators -> 1/x via ScalarE Ln+Exp per 512-query half
  (same ACT table set as Exp/Copy: no table switches), three of four halves
  hidden under the loop; broadcast via K=4 selector matmuls, one DVE multiply
  per pair, then cross-partition strip assembly into head-major channel order
  with SBUF->SBUF DMAs. Output projection consumes the assembled O^T; the
  proj bias is applied host-side; y is returned in bf16.

Exact algebraic simplifications vs the reference:
  - K bias dropped (softmax is invariant to per-query constant shifts)
  - V bias folded into the proj bias (softmax weights sum to 1, applied host-side)
  - head-dim scale folded into the exp's input scale / fast-exp multiplier

Environment workarounds (this walrus build): at most one sem wait per
instruction (waits hoisted onto injected NOPs), fp32/fp32r matmuls require
dst partition base 0, no gpsimd extended instructions (and GpSimd cannot
access PSUM), DMA cannot access PSUM or broadcast (zero-stride APs rejected).
"""

import sys

if "/opt/trn_rl_repo" not in sys.path:
    sys.path.insert(0, "/opt/trn_rl_repo")

import numpy as np

B = 8
C = 256
H = 8
HD = 32
MAXW = 1024
SCALE = HD ** -0.5
NEG_THRESH = -1e8  # mask values below this count as fully masked

_cached = {}


def _build_nc():
    import bass_rust
    import concourse.bass as bass
    import concourse.tile as tile
    import concourse.mybir as mybir
    from concourse.vector_clock import ScopedClock

    # ---- workaround: this walrus build accepts at most ONE sem wait per
    # instruction ("Too many sync wait commands" in setupSyncWait). Tile
    # attaches multi-sem waits freely. Split: hoist all but the last wait of
    # every committed instruction onto injected same-engine NOPs, and split
    # the final drain the same way.
    _ctr = [0]

    def _hoist_excess_waits(tc_self, inst, orig_add):
        si = inst.sync_info
        if si is not None:
            waits = list(si.on_wait or [])
            if len(waits) > 1:
                for w in waits[:-1]:
                    _ctr[0] += 1
                    nop = mybir.InstNoOp(name=f"waitsplit-{_ctr[0]}")
                    nop.engine = inst.engine
                    nop.sync_info = bass_rust.SyncInfo(on_wait=[w], on_update=[])
                    orig_add(tc_self, nop)
                si.on_wait = waits[-1:]
        orig_add(tc_self, inst)

    if not getattr(tile.TileContext, "_waitsplit_patched", False):
        _orig_add_instruction = tile.TileContext._add_instruction

        def _split_add_instruction(self, inst):
            _hoist_excess_waits(self, inst, _orig_add_instruction)

        tile.TileContext._add_instruction = _split_add_instruction

        def _patched_drain_and_barrier(self, tick_clock, wait_clock):
            # Trimmed exit: drain + one barrier. Semaphore clears and the
            # second barrier are dropped — each NEFF execution reinitializes
            # semaphores at program start, so the ~6us of teardown they cost
            # buys nothing (re-execution verified against the first run).
            nc = self.nc
            d0 = nc.sync.drain()
            wait_clock.add_sem_waits(
                d0.ins, ScopedClock({None: tick_clock.global_clock})
            )
            si = d0.ins.sync_info
            waits = list(si.on_wait) if si is not None else []
            if len(waits) > 1:
                si.on_wait = waits[0:1]
                for w in waits[1:]:
                    dk = nc.sync.drain()
                    dk.ins.sync_info = bass_rust.SyncInfo(on_wait=[w], on_update=[])
            nc.all_engine_barrier()
            assert self.sems is not None
            popped = nc._tile_sem_poison_stack.pop()
            assert popped is self._sem_poison

        tile.TileContext._drain_and_barrier = _patched_drain_and_barrier
        tile.TileContext._waitsplit_patched = True

    dt = mybir.dt
    f32 = dt.float32
    f32r = dt.float32r
    i16 = dt.int16
    AF = mybir.ActivationFunctionType
    ALU = mybir.AluOpType

    nc = bass.Bass(
        "TRN2",
        target_bir_lowering=False,
        debug=False,
        num_devices=1,
        enable_asserts=False,
    )

    bf16 = dt.bfloat16

    xt_d = nc.dram_tensor("xt", [128, 2048], bf16, kind="ExternalInput").ap()
    qw_d = nc.dram_tensor("qw", [128, 1536], bf16, kind="ExternalInput").ap()
    qb_d = nc.dram_tensor("qb", [128, 2], f32, kind="ExternalInput").ap()
    pw_d = nc.dram_tensor("pw", [128, 512], bf16, kind="ExternalInput").ap()
    mask_d = nc.dram_tensor("mask", [128, 8], f32, kind="ExternalInput").ap()
    maskb_d = nc.dram_tensor("maskb", [128, 8], f32, kind="ExternalInput").ap()
    sel_d = nc.dram_tensor("sel", [4, 256], bf16, kind="ExternalInput").ap()
    yt_d = nc.dram_tensor("yt", [128, 2048], bf16, kind="ExternalOutput").ap()

    # Schraudolph fast-exp (DVE): bf16(bits) = int16(round(s*FE_A + maskb))
    # where FE_A = SCALE*log2(e)*2^7; int16 saturation maps masked scores to
    # 0x8000 = -0.0. Calibrated bias constant folded into maskb host-side.
    FE_A = float(SCALE * 1.4426950408889634 * 128.0)

    with tile.TileContext(nc) as tc:
        with (
            tc.tile_pool(name="const", bufs=1) as const_pool,
            tc.tile_pool(name="big", bufs=1) as big_pool,
            tc.tile_pool(name="pt", bufs=6) as pt_pool,
            tc.tile_pool(name="stage", bufs=4) as stage_pool,
            tc.tile_pool(name="norm", bufs=2) as norm_pool,
            tc.tile_pool(name="ps_s4a", bufs=1, space="PSUM") as ps_s4a,
            tc.tile_pool(name="ps_s4b", bufs=1, space="PSUM") as ps_s4b,
            tc.tile_pool(name="ps_ot", bufs=2, space="PSUM") as ps_ot,
        ):
            xt = const_pool.tile([128, 2048], bf16, tag="xt")
            qw = const_pool.tile([128, 1536], bf16, tag="qw")
            qb = const_pool.tile([128, 2], f32, tag="qb")
            pw = const_pool.tile([128, 512], bf16, tag="pw")
            mask = const_pool.tile([128, 8], f32, tag="mask")
            maskb = const_pool.tile([128, 8], f32, tag="maskb")
            sel = const_pool.tile([4, 256], bf16, tag="sel")

            # startup DMAs ordered by first consumer: the first qk tiles need
            # xt chunks 0/2 and qw cols [0:384]+[768:1152]; mask gates the
            # first exp. sync/scalar/gpsimd queues run in parallel.
            nc.gpsimd.dma_start(mask[:], mask_d)
            nc.gpsimd.dma_start(maskb[:], maskb_d)
            # qw slices in exact first-consumer order: qk0 needs [0:128]+
            # [768:896], qk2 needs [256:384]+[1024:1152]; tiny first transfers
            # shave the DMA-latency-bound startup chain
            nc.sync.dma_start(xt[:, 0:512], xt_d[:, 0:512])
            nc.scalar.dma_start(qw[:, 0:128], qw_d[:, 0:128])
            nc.sync.dma_start(xt[:, 1024:1536], xt_d[:, 1024:1536])
            nc.scalar.dma_start(qw[:, 768:896], qw_d[:, 768:896])
            nc.scalar.dma_start(qw[:, 256:384], qw_d[:, 256:384])
            nc.scalar.dma_start(qw[:, 1024:1152], qw_d[:, 1024:1152])
            nc.sync.dma_start(xt[:, 512:1024], xt_d[:, 512:1024])
            nc.scalar.dma_start(qw[:, 128:256], qw_d[:, 128:256])
            nc.scalar.dma_start(qw[:, 896:1024], qw_d[:, 896:1024])
            nc.sync.dma_start(xt[:, 1536:2048], xt_d[:, 1536:2048])
            nc.scalar.dma_start(qw[:, 384:768], qw_d[:, 384:768])
            nc.scalar.dma_start(qw[:, 1152:1536], qw_d[:, 1152:1536])
            # dependency-free ACT table warmup (forces the Exp table load now,
            # after the critical qw DMA triggers so it doesn't delay them)
            warm_in = const_pool.tile([1, 2], f32, tag="warm_in")
            nc.vector.memset(warm_in[:], 0.0)
            warm = const_pool.tile([1, 2], f32, tag="warm")
            nc.scalar.activation(warm[:], warm_in[:], AF.Exp, scale=0.0)
            nc.gpsimd.dma_start(qb[:], qb_d)
            nc.gpsimd.dma_start(sel[:], sel_d)
            nc.gpsimd.dma_start(pw[:], pw_d)

            qt = big_pool.tile([128, 2048], bf16, tag="qt")
            kt = big_pool.tile([128, 2048], bf16, tag="kt")
            # [part, kpos_blk, head, head_dim+1]; col 32 = the all-ones column
            # that makes each PV matmul emit the softmax denominator as an
            # extra psum row (stationary = V_h | ones -> out rows 0-31 = O^T
            # strip, row 32 = rowsum)
            va = big_pool.tile([128, 8, 8, 33], bf16, tag="va")
            otf = big_pool.tile([128, 2048], bf16, tag="otf")
            ytile = big_pool.tile([128, 2048], bf16, tag="ytile")
            nc.vector.memset(va[:, :, :, 32:33], 1.0)

            # ---------- qkv projections ----------
            # V first (PV needs all of it), then the Q/K tiles needed by the
            # first head group; the rest is emitted between the groups so the
            # PE fills ACT-bound gaps.
            def qk_tile(m, chunks=(0, 1), pool=None, ptag="ot"):
                pool = pool if pool is not None else ps_ot
                for c in chunks:
                    ps = pool.tile([128, 512], f32, tag=ptag, name=f"qk{m}{c}")
                    for t in range(2):
                        nc.tensor.matmul(
                            ps[:],
                            qw[:, 768 * t + 128 * m : 768 * t + 128 * (m + 1)],
                            xt[:, 1024 * t + 512 * c : 1024 * t + 512 * (c + 1)],
                            start=(t == 0),
                            stop=(t == 1),
                        )
                    if m < 2:
                        nc.vector.tensor_scalar_add(
                            qt[:, 1024 * m + 512 * c : 1024 * m + 512 * (c + 1)],
                            ps[:],
                            qb[:, m : m + 1],
                        )
                    else:
                        nc.vector.tensor_copy(
                            kt[:, 1024 * (m - 2) + 512 * c : 1024 * (m - 2) + 512 * (c + 1)],
                            ps[:],
                        )

            # V: out[kpos_block, cv] in bf16 (no bias; folded into proj bias)
            def v_tile(j, pool=None, ptag="ot"):
                pool = pool if pool is not None else ps_ot
                ps = pool.tile([128, 256], f32, tag=ptag, name=f"v{j}")
                for t in range(2):
                    nc.tensor.matmul(
                        ps[:],
                        xt[:, 1024 * t + 128 * j : 1024 * t + 128 * (j + 1)],
                        qw[:, 768 * t + 512 : 768 * t + 768],
                        start=(t == 0),
                        stop=(t == 1),
                    )
                nc.vector.tensor_copy(
                    va[:, j, :, 0:32],
                    ps[:].rearrange("p (h d) -> p h d", d=32),
                )

            # only the chunk-0 Q/K tiles gate the first score matmul; the rest
            # of the projections are emitted inside the first iterations
            qk_tile(0, chunks=(0,))
            qk_tile(2, chunks=(0,))

            # ---------- attention: 4 heads (one group) at a time ----------
            # scores: 4-way row-packed fp32r matmuls, two [128,1024] psum tiles
            # per step (double-buffered so ScalarE exps run back-to-back), one
            # exp per tile -> bf16 P^T, then 8 bf16 PV/rowsum matmuls col-tiled
            # across the 4 PE column strips.
            # O^T strips: head hh at psum rows 32*hh of its pair psum
            # (pair 0 = hh 0,1 rows 0-63; pair 1 = hh 2,3 rows 64-127);
            # denominator rows: hh -> (64, 96, 0, 32).
            saved = {}
            for grp in range(2):
                ov0 = ps_ot.tile([128, 1024], f32, tag="ot", name=f"ov0_{grp}")
                ov1 = ps_ot.tile([128, 1024], f32, tag="ot", name=f"ov1_{grp}")

                def emit_scores(j, c):
                    s4a = ps_s4a.tile([128, 1024], f32, tag="s4a", name=f"s4a{grp}{j}{c}")
                    s4b = ps_s4b.tile([128, 1024], f32, tag="s4b", name=f"s4b{grp}{j}{c}")
                    for hh in range(4):
                        s4 = s4a if hh < 2 else s4b
                        base = 32 * hh
                        nc.tensor.matmul(
                            s4[:, 512 * (hh % 2) : 512 * (hh % 2 + 1)],
                            kt[base : base + 32,
                               1024 * grp + 128 * j : 1024 * grp + 128 * (j + 1)],
                            qt[base : base + 32,
                               1024 * grp + 512 * c : 1024 * grp + 512 * (c + 1)],
                            start=True,
                            stop=True,
                            tile_position=(base, 0),
                        )
                    pta = pt_pool.tile([128, 1024], bf16, tag="pt", name=f"pta{grp}{j}{c}")
                    ptb = pt_pool.tile([128, 1024], bf16, tag="pt", name=f"ptb{grp}{j}{c}")
                    nc.scalar.activation(
                        pta[:], s4a[:], AF.Exp, bias=mask[:, j : j + 1], scale=SCALE,
                    )
                    # heads 2,3 of the group take the DVE fast-exp: halves the
                    # ScalarE stream and releases both score psums in parallel
                    # so all four score matmuls can row-pack concurrently
                    nc.vector.tensor_scalar(
                        ptb[:].bitcast(i16), s4b[:], FE_A, maskb[:, j : j + 1],
                        ALU.mult, ALU.add,
                    )
                    return pta, ptb

                def emit_pv(pta, ptb, j, c):
                    sj = (j == 0)
                    ej = (j == 7)
                    # fused PV+rowsum: stationary is V_h | ones [128, 33], so
                    # each head's matmul emits its O^T strip (rows 0-31 of the
                    # 33-row block) AND its softmax denominator (row 32) from a
                    # single moving stream. 33 output rows round up to a 64-col
                    # PE tile, so the two heads of a pair sit at tile columns 0
                    # and 64 (psum rows 0-32 / 64-96).
                    for hh in range(4):
                        h = 4 * grp + hh
                        ov = ov0 if hh < 2 else ov1
                        pt = pta if hh < 2 else ptb
                        vpos = 64 * (hh % 2)
                        nc.tensor.matmul(
                            ov[vpos : vpos + 33, 512 * c : 512 * (c + 1)],
                            va[:, j, h, :],
                            pt[:, 512 * (hh % 2) : 512 * (hh % 2 + 1)],
                            start=sj,
                            stop=ej,
                            tile_position=(0, vpos),
                        )

                st0 = stage_pool.tile([128, 1024], f32, tag="st", name=f"st0_{grp}")
                st1 = stage_pool.tile([128, 1024], f32, tag="st", name=f"st1_{grp}")
                se4 = norm_pool.tile([4, 1024], f32, tag="se4", name=f"se4_{grp}")
                rc4 = norm_pool.tile([4, 1024], bf16, tag="rc4", name=f"rc4_{grp}")

                def emit_half_tail(c):
                    # the c-major order finishes each 512-query half's PV
                    # accumulation 8 iterations before the grp ends: stage it
                    # and extract its denominators while the loop continues.
                    # With the fused PV+rowsum, denominators live at psum rows
                    # 32 (even head) and 96 (odd head) of each pair psum.
                    sl = slice(512 * c, 512 * (c + 1))
                    last = grp == 1 and c == 1
                    # staging split across ScalarE and DVE so neither exp
                    # stream takes the full 1.4us boundary insert
                    nc.scalar.copy(st0[:, sl], ov0[:, sl])
                    nc.vector.tensor_copy(st1[:, sl], ov1[:, sl])
                    nc.sync.dma_start(se4[0:1, sl], st0[32:33, sl])
                    nc.gpsimd.dma_start(se4[1:2, sl], st0[96:97, sl])
                    nc.sync.dma_start(se4[2:3, sl], st1[32:33, sl])
                    nc.scalar.dma_start(se4[3:4, sl], st1[96:97, sl])
                    if not last:
                        # reciprocal for this half on ScalarE (Ln + Exp share
                        # the loaded table set): ~1.3us riding the exp stream's
                        # slack, so the exit tail only handles the final half
                        lnh = norm_pool.tile([4, 512], f32, tag="ln4", name=f"ln_{grp}{c}")
                        nc.scalar.activation(lnh[:], se4[:, sl], AF.Ln)
                        nc.scalar.activation(rc4[:, sl], lnh[:], AF.Exp, scale=-1.0)

                iters = [(j, c) for c in range(2) for j in range(8)]
                pend = None
                for idx in range(len(iters) + 1):
                    if idx < len(iters):
                        j, c = iters[idx]
                        cur = (*emit_scores(j, c), j, c)
                        if grp == 0 and idx == 0:
                            # V projections and the remaining Q/K tiles fill the
                            # first exp latencies; emitted before any PV so the
                            # ov accumulators don't yet hold the psum slots
                            v_tile(0)
                            v_tile(1)
                            for jj in range(2, 6):
                                v_tile(jj)
                        if grp == 0 and idx == 1:
                            v_tile(6)
                            v_tile(7)
                            qk_tile(0, chunks=(1,))
                            qk_tile(2, chunks=(1,))
                            qk_tile(1)
                            qk_tile(3)
                    else:
                        cur = None
                    if pend is not None:
                        emit_pv(*pend)
                        if pend[2] == 7:
                            emit_half_tail(pend[3])
                    pend = cur

                # ---- last-half reciprocal (the other halves ran mid-loop) ----
                if grp == 1:
                    ln4 = norm_pool.tile([4, 512], f32, tag="ln4", name=f"ln4_{grp}")
                    nc.scalar.activation(ln4[:], se4[:, 512:1024], AF.Ln)
                    nc.scalar.activation(rc4[:, 512:1024], ln4[:], AF.Exp, scale=-1.0)
                saved[grp] = (st0, st1, rc4)

            # ---- deferred normalization: broadcast 1/denominator and scale ----
            # bca rows 0-31 = 1/d(h_even), rows 64-95 = 1/d(h_odd) (matches the
            # staged psum layout); bcb likewise for the st1 pair. The nx tiles
            # are then strip-assembled into otf's head-major channel order with
            # cross-partition SBUF->SBUF DMAs (DMA is the only engine that can
            # move data across partitions). grp1 is split per 512-query half so
            # the c0 chain (reciprocal precomputed mid-loop) drains while the
            # c1 chain waits on the post-loop Ln+Exp.
            bct = {}
            ntiles = {}
            for grp in range(2):
                st0, st1, rc4 = saved[grp]
                bca = ps_s4a.tile([128, 1024], f32, tag="s4a", name=f"bca{grp}")
                bcb = ps_s4b.tile([128, 1024], f32, tag="s4b", name=f"bcb{grp}")
                bct[grp] = (bca, bcb)
                ntiles[grp] = (
                    pt_pool.tile([128, 1024], bf16, tag="pt", name=f"n0_{grp}"),
                    pt_pool.tile([128, 1024], bf16, tag="pt", name=f"n1_{grp}"),
                )

            def emit_norm(grp, cs):
                st0, st1, rc4 = saved[grp]
                bca, bcb = bct[grp]
                n0, n1 = ntiles[grp]
                sl = slice(512 * cs[0], 512 * (cs[-1] + 1))
                for c in cs:
                    csl = slice(512 * c, 512 * (c + 1))
                    nc.tensor.matmul(
                        bca[:, csl], sel[:, 0:128], rc4[:, csl], start=True, stop=True
                    )
                    nc.tensor.matmul(
                        bcb[:, csl], sel[:, 128:256], rc4[:, csl], start=True, stop=True
                    )
                nc.vector.tensor_mul(n0[:, sl], st0[:, sl], bca[:, sl])
                nc.vector.tensor_mul(n1[:, sl], st1[:, sl], bcb[:, sl])
                gb = slice(1024 * grp + sl.start, 1024 * grp + sl.stop)
                # three parallel trigger queues so all four strips land ~together
                eng0 = nc.sync if grp == 0 else nc.scalar
                eng0.dma_start(otf[0:32, gb], n0[0:32, sl])
                nc.gpsimd.dma_start(otf[32:64, gb], n0[64:96, sl])
                nc.sync.dma_start(otf[64:96, gb], n1[0:32, sl])
                nc.gpsimd.dma_start(otf[96:128, gb], n1[64:96, sl])

            def emit_proj(c):
                for m in range(2):
                    ps = ps_ot.tile([128, 512], f32, tag="ot")
                    for t in range(2):
                        nc.tensor.matmul(
                            ps[:],
                            pw[:, 256 * t + 128 * m : 256 * t + 128 * (m + 1)],
                            otf[:, 1024 * t + 512 * c : 1024 * t + 512 * (c + 1)],
                            start=(t == 0),
                            stop=(t == 1),
                        )
                    ysl = slice(1024 * m + 512 * c, 1024 * m + 512 * (c + 1))
                    # proj bias is applied host-side; c1 copies ride the
                    # by-then-idle ScalarE so DVE isn't the tail bottleneck
                    (nc.vector.tensor_copy if c == 0 else nc.scalar.copy)(
                        ytile[:, ysl], ps[:]
                    )
                    # all output stores ride the sync queue: by proj time it is
                    # idle, whereas gpsimd still drains the c1 assembly
                    # transfers ahead of the final store that gates the drain
                    nc.sync.dma_start(yt_d[:, ysl], ytile[:, ysl])

            emit_norm(0, (0, 1))
            emit_norm(1, (0,))
            emit_norm(1, (1,))
            emit_proj(0)
            emit_proj(1)

    return nc


def _get_nc():
    if "nc" not in _cached:
        _cached["nc"] = _build_nc()
    return _cached["nc"]


def _pack_per_partition(a2d):
    """[2*128, F] -> [128, 2*F] with tile t at cols F*t."""
    n, f = a2d.shape
    t = n // 128
    return np.ascontiguousarray(
        a2d.reshape(t, 128, f).transpose(1, 0, 2).reshape(128, t * f)
    )


def _prepare(carrier_tokens, ct_mask, batch_num_windows, qkv_w, qkv_b, proj_w, proj_b):
    """Host-side bookkeeping: ragged->padded scatter, weight packing.
    Returns (in_maps, ctx) where ctx carries what postprocessing needs."""
    carrier_tokens = np.asarray(carrier_tokens, dtype=np.float32)
    ct_mask = np.asarray(ct_mask, dtype=np.float32)
    lens = np.asarray(batch_num_windows).astype(np.int64)
    qkv_w = np.asarray(qkv_w, dtype=np.float32)
    qkv_b = np.asarray(qkv_b, dtype=np.float32)
    proj_w = np.asarray(proj_w, dtype=np.float32)
    proj_b = np.asarray(proj_b, dtype=np.float32)

    total = carrier_tokens.shape[0]

    # ragged -> padded bookkeeping (mirrors the reference's scatter semantics:
    # OOB scatter indices dropped, OOB gather indices clipped)
    offsets = np.concatenate([[0], np.cumsum(lens)])
    tok = np.arange(total)
    b_id = np.searchsorted(offsets[1:], tok, side="right")
    w_id = tok - offsets[np.minimum(b_id, B)]
    flat_idx = b_id * MAXW + w_id
    valid = flat_idx < B * MAXW
    padded = np.zeros((B * MAXW, C), np.float32)
    padded[flat_idx[valid]] = carrier_tokens[valid]
    padded = padded.reshape(B, MAXW, C)

    mask_col = np.ascontiguousarray(ct_mask[:, 0, :])  # [B, MAXW]

    # host-side exact weight transforms; the effective proj bias (V bias
    # folded in via softmax-sums-to-1) is applied host-side in _postprocess
    pb_eff = qkv_b[2 * C : 3 * C] @ proj_w + proj_b

    qw_packed = _pack_per_partition(qkv_w)                      # [128, 1536]
    qb_packed = np.ascontiguousarray(qkv_b[0:C].reshape(2, 128).T)
    pw_packed = _pack_per_partition(proj_w)                     # [128, 512]

    import ml_dtypes
    # sel[:, 0:128] broadcasts 1/d of the even/odd head of the st0 pair to
    # rows 0-31 / 64-95; sel[:, 128:256] likewise for the st1 pair
    sel_arr = np.zeros((4, 256), ml_dtypes.bfloat16)
    sel_arr[0, 0:32] = 1.0
    sel_arr[1, 64:96] = 1.0
    sel_arr[2, 128:160] = 1.0
    sel_arr[3, 192:224] = 1.0
    qw_packed = qw_packed.astype(ml_dtypes.bfloat16)
    pw_packed = pw_packed.astype(ml_dtypes.bfloat16)
    # fast-exp per-partition bias: maskb = mask*log2(e)*2^7 + (2^7*127 - c)
    # with the calibrated correction c = 11.5 (int16 round-to-nearest)
    LOG2E = 1.4426950408889634
    FE_B0 = np.float32(16256.0 - 11.5)
    in_maps = []
    for b in range(B):
        xt = _pack_per_partition(padded[b].T).astype(ml_dtypes.bfloat16)
        mb = np.ascontiguousarray(mask_col[b].reshape(8, 128).T)
        mbb = (mb * np.float32(LOG2E * 128.0) + FE_B0).astype(np.float32)
        in_maps.append(
            {
                "xt": xt,
                "qw": qw_packed,
                "qb": qb_packed,
                "pw": pw_packed,
                "mask": mb,
                "maskb": mbb,
                "sel": sel_arr,
            }
        )

    ctx = {
        "flat_idx": flat_idx,
        "mask_col": mask_col,
        "padded": padded,
        "qkv_w": qkv_w,
        "qkv_b": qkv_b,
        "proj_w": proj_w,
        "proj_b": proj_b,
        "pb_eff": pb_eff,
    }
    return in_maps, ctx


def _postprocess(results, ctx):
    """Per-core outputs -> full ragged output (gather + degenerate-row fix)."""
    flat_idx = ctx["flat_idx"]
    mask_col = ctx["mask_col"]
    padded = ctx["padded"]
    qkv_w, qkv_b = ctx["qkv_w"], ctx["qkv_b"]
    proj_w, proj_b = ctx["proj_w"], ctx["proj_b"]

    y_pad = np.empty((B, MAXW, C), np.float32)
    for b in range(B):
        yt = np.asarray(results[b]["yt"], dtype=np.float32)     # [128, 2048]
        y_t = yt.reshape(128, 2, MAXW).transpose(1, 0, 2).reshape(C, MAXW)
        y_pad[b] = y_t.T
    y_flat = y_pad.reshape(B * MAXW, C)
    gather_idx = np.clip(flat_idx, 0, B * MAXW - 1)
    # the effective proj bias is applied here (host side) rather than on-chip
    out = y_flat[gather_idx] + ctx["pb_eff"].astype(np.float32)[None, :]

    # degenerate rows: gathered positions whose key mask is fully masked.
    # The reference's softmax (with max-subtraction) gives uniform weights
    # there; our exp underflows to 0/0. Recompute those rows exactly.
    row_b = np.minimum(gather_idx // MAXW, B - 1)
    degenerate_batches = [b for b in range(B) if np.all(mask_col[b] < NEG_THRESH)]
    for b in degenerate_batches:
        rows = np.nonzero(row_b == b)[0]
        if rows.size == 0:
            continue
        vmat = padded[b] @ qkv_w[:, 2 * C : 3 * C] + qkv_b[2 * C : 3 * C]
        mean_v = vmat.mean(axis=0)  # uniform attention, same for all heads
        fix = mean_v @ proj_w + proj_b
        out[rows] = fix.astype(np.float32)

    return np.ascontiguousarray(out.astype(np.float32))


def run_device(in_maps, **spmd_kwargs):
    from concourse import bass_utils

    nc = _get_nc()
    return bass_utils.run_bass_kernel_spmd(
        nc, in_maps, core_ids=list(range(B)), **spmd_kwargs
    )


def kernel(carrier_tokens, ct_mask, batch_num_windows, qkv_w, qkv_b, proj_w, proj_b):
    in_maps, ctx = _prepare(
        carrier_tokens, ct_mask, batch_num_windows, qkv_w, qkv_b, proj_w, proj_b
    )
    res = run_device(in_maps, trace=False)
    return _postprocess(res.results, ctx)

